# revision 53
# baseline (speedup 1.0000x reference)
"""Trainium2 Bass kernel for nn_CausalMixer (QMIX-style causal mixer).

Data-parallel across 8 NeuronCores: batch dim sharded round-robin
(core m gets batches m, m+8, m+16, ...), hypernet weights replicated.

Per-core layout (R = 1024 rows = 16 batches x 64 timesteps):
  - stage-1 "transposed" GEMMs: out[feat, rows] = Wcat.T-chunks @ states.T,
    evicted with fused per-partition bias+ReLU (alternating ScalarE / DVE).
  - stage-2 row-major GEMMs: the relu'd z tiles [feat, rows] serve directly
    as lhsT, producing per-row hypernet weights [rows, feat]; bias is
    preloaded into PSUM with a K=1 ones-matmul.
  - gather (qvals[cr]) precomputed on host, shipped in the mega tensor.
  - the onehot quirk (batch row b==v gets +delta) is handled as a rank-1
    correction on chunk 0 only (host orders the diag batches first).
  - all bf16 inputs ride one mega-packed DRAM tensor split across the two
    HW DGE queues (SP + Activation), critical (wcat,states) pairs first.

Scheduling (tuned against perfetto traces):
  - a 10-matmul K=128/N=512 warmup train on a memset tile runs from kernel
    start so the HAM clock gate flips to 8/8 (~2.4 GHz) before the real
    GEMMs; K=1 ones-matmuls do NOT register as array activity.
  - the s1t passes run kc-major across all 6 feature chunks (6 concurrent
    PSUM banks), so each arriving (wcat|s_t) DMA pair unlocks 6 matmuls
    and the PE FIFO never head-of-line blocks on a late pair; kc 2,3 then
    go fc-by-fc (z1 first) so evictions start early.
  - everything downstream is split into batch-halves (rc 0-3 / 4-7) and
    pipelined behind the PE stream; sparse N=4/N=32 blocks are sandwiched
    between dense w1 trains to keep HAM array-duty up; the final
    elu/y blocks are consolidated (0..5 and 6..7) with per-half output
    DMAs so only the last two chunks gate the kernel end.
"""

import sys

for _p in ("/root/.axon_site", "/root/.axon_site/_ro/trn_rl_repo",
           "/root/.axon_site/_ro/pypackages", "/opt/trn_rl_repo"):
    if _p not in sys.path:
        sys.path.append(_p)

import numpy as np
import ml_dtypes
from contextlib import ExitStack

import concourse.bass as bass
import concourse.bacc as bacc_mod
import concourse.tile as tile
import concourse.mybir as mybir
from concourse.bass import broadcast_tensor_aps
from concourse.bass_utils import run_bass_kernel_spmd

BF = ml_dtypes.bfloat16
DT = mybir.dt.bfloat16
F32 = mybir.dt.float32
OP = mybir.AluOpType
AF = mybir.ActivationFunctionType
AX = mybir.AxisListType

NCORES = 8
B, T, NA, NV, K, SD, H, E = 128, 64, 10, 16, 4, 512, 256, 32
R = 16 * T            # rows per core = 1024
C = R // 128          # row chunks per core = 8
NVK = NV * K          # 64
JW = NVK + K          # 68 gather cols (64 + 4 diag)
SMALL = NA + 1 + 1 + E + E   # 76: w01 | b01 | b00 | b1 | b2_l1
W1COL = (NV + 1) * E  # 544

# ---- mega-packed bf16 input column map (need-ordered) -------------------
# gat 8x68 | qvb 8x10, then 4x [wcat_kc (768) | s_t_kc (1024)] pairs, then:
#   wsmall 4x76 | w1l2 2x544 | w2l2 2x32 | w0l2 2x4
#   | brow_small4 4x76 | brow_w1 544 | brow_w20 36 | brow_w2x2 64
#   | brow_w0x2 8  (partition 0)
PAIR = 768 + R
OFF_CRX = 0
OFF_QVB = OFF_CRX + C * JW
OFF_PAIR = OFF_QVB + C * NA
OFF_TAILA = OFF_PAIR + 4 * PAIR
OFF_WSMALL = OFF_TAILA
OFF_W0L2 = OFF_WSMALL + 4 * SMALL
OFF_BROW = OFF_W0L2 + 2 * K
OFF_BROW2 = OFF_BROW + 4 * SMALL + W1COL + E + K
OFF_TAILB = OFF_BROW2 + 2 * E + 2 * K
OFF_W1L2 = OFF_TAILB
OFF_W2L2 = OFF_W1L2 + 2 * W1COL
NBF = OFF_W2L2 + 2 * E
# f32 mega: bias_t (8) | dmask (16) | consts (34)
GOFF_BIAS = 0
GOFF_DMASK = 8
GOFF_CONSTS = 24
GOFF_BIASN = GOFF_CONSTS + E + 2
NF32 = GOFF_BIASN + 8

_cache = {}


def _build_nc():
    nc = bacc_mod.Bacc("TRN2", target_bir_lowering=False, debug=False)

    mb_d = nc.dram_tensor("mb", [128, NBF], DT, kind="ExternalInput")
    mf_d = nc.dram_tensor("mf", [128, NF32], F32, kind="ExternalInput")
    out_d = nc.dram_tensor("out", [128, C], F32, kind="ExternalOutput")

    with tile.TileContext(nc) as tc, ExitStack() as ctx:
        pool = ctx.enter_context(tc.tile_pool(name="sbuf", bufs=1))
        hpool = ctx.enter_context(tc.tile_pool(name="hbuf", bufs=3))
        psum = ctx.enter_context(tc.tile_pool(name="psum", bufs=2, space="PSUM"))

        mb_s = pool.tile([128, NBF], DT)
        mf_s = pool.tile([128, NF32], F32)

        # ---- DMAs on both HW DGE queues; (wcat | s_t rh0) pairs first ----
        def dma_cols(eng, c0, c1):
            eng.dma_start(mb_s[:, c0:c1], mb_d[:, c0:c1])

        nc.scalar.dma_start(mf_s[:], mf_d[:])
        dma_cols(nc.sync, OFF_PAIR + 0 * PAIR, OFF_PAIR + 0 * PAIR + 1280)
        dma_cols(nc.scalar, OFF_PAIR + 1 * PAIR, OFF_PAIR + 1 * PAIR + 1280)
        dma_cols(nc.sync, OFF_PAIR + 2 * PAIR, OFF_PAIR + 2 * PAIR + 1280)
        dma_cols(nc.scalar, OFF_PAIR + 3 * PAIR, OFF_PAIR + 3 * PAIR + 1280)
        dma_cols(nc.sync, OFF_TAILB, NBF)         # w1l2 | w2l2 (w1 blocks)
        dma_cols(nc.scalar, OFF_TAILA, OFF_TAILB)  # wsmall | w0l2 | bias rows
        dma_cols(nc.sync, OFF_PAIR + 0 * PAIR + 1280, OFF_PAIR + 1 * PAIR)
        dma_cols(nc.scalar, OFF_PAIR + 1 * PAIR + 1280, OFF_PAIR + 2 * PAIR)
        dma_cols(nc.sync, OFF_PAIR + 2 * PAIR + 1280, OFF_PAIR + 3 * PAIR)
        dma_cols(nc.scalar, OFF_CRX, OFF_PAIR)     # gat | qvb (group chain)
        dma_cols(nc.scalar, OFF_PAIR + 3 * PAIR + 1280, OFF_PAIR + 4 * PAIR)

        def wcat(kc, c0, c1):
            return mb_s[:, OFF_PAIR + kc * PAIR + c0:OFF_PAIR + kc * PAIR + c1]

        def s_t(kc, c0, c1):
            return mb_s[:, OFF_PAIR + kc * PAIR + 768 + c0:
                        OFF_PAIR + kc * PAIR + 768 + c1]

        def wsmall(kc):
            return mb_s[:, OFF_WSMALL + kc * SMALL:OFF_WSMALL + (kc + 1) * SMALL]

        def w1l2(kc, c0, c1):
            return mb_s[:, OFF_W1L2 + kc * W1COL + c0:OFF_W1L2 + kc * W1COL + c1]

        def w2l2(kc):
            return mb_s[:, OFF_W2L2 + kc * E:OFF_W2L2 + (kc + 1) * E]

        def w0l2(kc):
            return mb_s[:, OFF_W0L2 + kc * K:OFF_W0L2 + (kc + 1) * K]

        acc_s = mb_s[:, OFF_CRX:OFF_CRX + C * JW].rearrange(
            "p (c j) -> p c j", j=JW)          # host-gathered q-values
        qvb_s = mb_s[:, OFF_QVB:OFF_QVB + C * NA].rearrange(
            "p (c j) -> p c j", j=NA)
        brow_small4_s = mb_s[0:1, OFF_BROW:OFF_BROW + 4 * SMALL]
        brow_w1_s = mb_s[0:1, OFF_BROW + 4 * SMALL:OFF_BROW + 4 * SMALL + W1COL]
        brow_w0_s = mb_s[0:1, OFF_BROW + 4 * SMALL + W1COL + E:
                         OFF_BROW + 4 * SMALL + W1COL + E + K]
        brow_w2x2_s = mb_s[0:1, OFF_BROW2:OFF_BROW2 + 2 * E]
        brow_w0x2_s = mb_s[0:1, OFF_BROW2 + 2 * E:OFF_BROW2 + 2 * E + 2 * K]
        bias_t_s = mf_s[:, GOFF_BIAS:GOFF_BIAS + 8]
        dmask_s = mf_s[:, GOFF_DMASK:GOFF_DMASK + NV]
        consts_s = mf_s[:, GOFF_CONSTS:GOFF_CONSTS + E + 2]
        biasn_s = mf_s[:, GOFF_BIASN:GOFF_BIASN + 8]

        ones_s = pool.tile([1, 512], DT)
        nc.gpsimd.memset(ones_s[:], 1.0)

        # ---- PE warmup: HAM counts *array* activity, so K=1 ones-matmuls
        # do NOT lift the clock gate — use full K=128 matmuls on a memset
        # tile. The train bridges from kernel start until the first
        # wcat/s_t pair lands (~12us) so the real s1t GEMMs run at 2.4 GHz.
        def bc(ap, like):
            a, _ = broadcast_tensor_aps(ap, like)
            return a

        warm_s = pool.tile([128, 512], DT)
        nc.gpsimd.memset(warm_s[:], 0.0)
        for i in range(10):
            pw = psum.tile([128, 512], F32, tag="s1r", bufs=2)
            nc.tensor.matmul(pw[:], warm_s[:, 0:128], warm_s[:],
                             start=True, stop=True)

        # ---- stage-1 transposed GEMMs: z = relu(Wcat.T @ states + b) ----
        z_s = pool.tile([128, 6, R], DT)      # zA | z1 | z2 (feat-major)
        zad_s = pool.tile([128, 2, 128], DT)  # diag zA, chunk 0 only

        def s1t_evict(p1, fc, rh):
            dst = z_s[:, fc, rh * 512:(rh + 1) * 512]
            # rh1: bias toward scalar (DVE only takes fc 3, 1) — the DVE
            # FIFO carries the mix chain for rc 0-3 in that window.
            on_dve = (fc % 2 == 0) if rh == 0 else (fc in (3, 1))
            if on_dve:
                nc.vector.scalar_tensor_tensor(
                    dst, p1[:], biasn_s[:, fc:fc + 1],
                    bc(bias_t_s[:, fc:fc + 1], p1[:]),
                    OP.max, OP.add)
            else:
                nc.scalar.activation(dst, p1[:], AF.Relu,
                                     bias=bias_t_s[:, fc:fc + 1])
            if rh == 0 and fc < 2:
                nc.scalar.activation(zad_s[:, fc, :], p1[:, 0:128],
                                     AF.Relu,
                                     bias=bias_t_s[:, 6 + fc:7 + fc])

        # kc-major over all 6 feature chunks: each arriving (wcat, s_t) DMA
        # pair unlocks 6 matmuls, so the PE FIFO never head-of-line blocks
        # on a late pair. kc 2,3 then go fc-by-fc so evictions start early,
        # prioritising z1 (fc 2,3 -> w1 blocks) then z00 (group chain).
        def s1t_alloc(rh):
            return [psum.tile([128, 512], F32, tag="s1t", bufs=6,
                              name=f"p1_{rh}_{fc}") for fc in range(6)]

        def s1t_round(rh, ps, kc):
            for fc in range(6):
                nc.tensor.matmul(
                    ps[fc][:], wcat(kc, fc * 128, (fc + 1) * 128),
                    s_t(kc, rh * 512, (rh + 1) * 512),
                    start=(kc == 0), stop=False)

        def s1t_kc23(rh, ps, extras=None):
            # extras: {fc: callable} — sparse N=4/8 blocks woven between the
            # dense N=512 groups so no HAM window drops below the duty gate.
            for fc in (2, 3, 0, 1, 4, 5):
                for kc in (2, 3):
                    nc.tensor.matmul(
                        ps[fc][:], wcat(kc, fc * 128, (fc + 1) * 128),
                        s_t(kc, rh * 512, (rh + 1) * 512),
                        start=False, stop=(kc == 3))
                s1t_evict(ps[fc], fc, rh)
                if extras and fc in extras:
                    extras[fc]()

        # ---- stage-1 row-major small heads, kc-grouped so the N=76 runs
        # interleave with the dense s1t rounds (each small-kc group needs
        # exactly the DMA pair the preceding s1t round consumed, and the
        # alternation keeps HAM array-duty above the re-throttle point).
        small_s = pool.tile([128, C, SMALL], F32)  # w01|b01|b00|b1|zb2(pre-relu)

        def small_preload(g):
            p2 = psum.tile([128, 4, SMALL], F32, tag="s1r", bufs=2)
            nc.tensor.matmul(p2[:].rearrange("p a b -> p (a b)"),
                             ones_s[:, 0:128], brow_small4_s,
                             start=True, stop=False)
            return p2

        def small_kc(g, p2, kc):
            for rl in range(4):
                rc = g * 4 + rl
                nc.tensor.matmul(p2[:, rl, :],
                                 s_t(kc, rc * 128, (rc + 1) * 128),
                                 wsmall(kc), start=False, stop=(kc == 3))
            if kc == 3:
                nc.scalar.copy(small_s[:, g * 4:(g + 1) * 4, :], p2[:])

        # ---- w0c / w0d (needs z00 only; unblocks the group chain) -------
        w0c_s = pool.tile([128, C, K], DT)
        w0d_s = pool.tile([128, K], DT)       # |w0_diag|, chunk 0

        def w0c_block(rcps):
            for rcp in rcps:
                p4 = psum.tile([128, 2, K], F32, tag="s1r", bufs=2)
                nc.tensor.matmul(p4[:].rearrange("p a b -> p (a b)"),
                                 ones_s[:, 0:128], brow_w0x2_s,
                                 start=True, stop=False)
                for rl in range(2):
                    rc = rcp * 2 + rl
                    for kc in range(2):
                        nc.tensor.matmul(p4[:, rl, :],
                                         z_s[:, 0 + kc, rc * 128:(rc + 1) * 128],
                                         w0l2(kc), start=False, stop=(kc == 1))
                nc.scalar.activation(w0c_s[:, rcp * 2:rcp * 2 + 2, :],
                                     p4[:], AF.Abs)

        def w0d_block():
            p6 = psum.tile([128, K], F32, tag="s1r", bufs=2)
            nc.tensor.matmul(p6[:], ones_s[:, 0:128], brow_w0_s,
                             start=True, stop=False)
            for kc in range(2):
                nc.tensor.matmul(p6[:], zad_s[:, kc, :], w0l2(kc),
                                 start=False, stop=(kc == 1))
            nc.scalar.activation(w0d_s[:], p6[:], AF.Abs)

        # ---- b2 head (needs only small_s / consts), per batch-half ------
        zb2r_s = pool.tile([128, C, E], F32)
        b2p_s = pool.tile([128, C, E], F32)
        b2v_s = pool.tile([128, C], F32)
        cb2 = consts_s[:, 0:E].rearrange("p (o e) -> p o e", o=1)

        def b2_head(g):
            cs = slice(g * 4, (g + 1) * 4)
            nc.scalar.activation(zb2r_s[:, cs, :], small_s[:, cs, 44:76],
                                 AF.Relu)
            nc.gpsimd.tensor_tensor(b2p_s[:, cs, :], zb2r_s[:, cs, :],
                                    bc(cb2, zb2r_s[:, cs, :]), OP.mult)
            nc.vector.tensor_reduce(b2v_s[:, cs], b2p_s[:, cs, :],
                                    AX.X, OP.add)

        # ---- group values + "other" head + gq assembly, per half --------
        gath4 = acc_s[:, :, 0:NVK].rearrange("p c (v k) -> p c v k", k=K)
        w04 = w0c_s.rearrange("p c (o k) -> p c o k", o=1)
        prodg_s = pool.tile([128, C, NV, K], DT)
        group_s = pool.tile([128, C, NV], F32)
        prodo_s = pool.tile([128, C, NA], F32)
        other_s = pool.tile([128, C], F32)
        gq_s = pool.tile([128, C, NV + 1], DT)

        def group_half(g):
            cs = slice(g * 4, (g + 1) * 4)
            nc.vector.tensor_tensor(prodg_s[:, cs], gath4[:, cs],
                                    bc(w04[:, cs], gath4[:, cs]), OP.mult)
            nc.vector.tensor_reduce(group_s[:, cs], prodg_s[:, cs],
                                    AX.X, OP.add)
            gb = small_s[:, cs, 11:12]
            nc.vector.tensor_tensor(group_s[:, cs], group_s[:, cs],
                                    bc(gb, group_s[:, cs]), OP.add)
            if g == 0:
                # diag correction (chunk 0 only)
                dw_s = pool.tile([128, K], F32)
                nc.vector.tensor_tensor(dw_s[:], w0d_s[:], w0c_s[:, 0, :],
                                        OP.subtract)
                gselp_s = pool.tile([128, K], F32)
                nc.vector.tensor_tensor(gselp_s[:], acc_s[:, 0, NVK:JW],
                                        dw_s[:], OP.mult)
                corr0_s = pool.tile([128, 1], F32)
                nc.vector.tensor_reduce(corr0_s[:], gselp_s[:], AX.X, OP.add)
                corr_s = pool.tile([128, 1], F32)
                nc.vector.tensor_scalar(corr_s[:], corr0_s[:],
                                        consts_s[:, E:E + 1], None, OP.add)
                nc.vector.scalar_tensor_tensor(group_s[:, 0, :], dmask_s,
                                               corr_s[:], group_s[:, 0, :],
                                               OP.mult, OP.add)
            nc.vector.tensor_tensor(prodo_s[:, cs], qvb_s[:, cs],
                                    small_s[:, cs, 0:NA], OP.mult)
            nc.vector.tensor_reduce(other_s[:, cs], prodo_s[:, cs],
                                    AX.X, OP.add)
            nc.vector.tensor_tensor(other_s[:, cs], other_s[:, cs],
                                    small_s[:, cs, NA], OP.add)
            nc.gpsimd.tensor_copy(gq_s[:, cs, 0:NV], group_s[:, cs])
            nc.gpsimd.tensor_copy(
                gq_s[:, cs, NV:NV + 1],
                other_s[:, cs].rearrange("p (c o) -> p c o", o=1))


        # ---- stage-2 w1 GEMMs + per-rc |w1| evictions + gq mix ----------
        # w1r cols are e-major (col = e*17+v); mix[p,rc,e] = sum_v gq*|w1|
        w1r_s = pool.tile([128, C, W1COL], DT)
        mix_s = pool.tile([128, C, E], F32)

        def w1_block(rcs):
            for rc in rcs:
                p3h = []
                for h in range(2):
                    p3 = psum.tile([128, 272], F32, tag="s1t", bufs=6, name="p3")
                    p3h.append(p3)
                    nc.tensor.matmul(p3[:], ones_s[:, 0:128],
                                     brow_w1_s[:, h * 272:(h + 1) * 272],
                                     start=True, stop=False)
                for kc in range(2):
                    for h in range(2):
                        nc.tensor.matmul(p3h[h][:],
                                         z_s[:, 2 + kc, rc * 128:(rc + 1) * 128],
                                         w1l2(kc, h * 272, (h + 1) * 272),
                                         start=False, stop=(kc == 1))
                nc.scalar.activation(w1r_s[:, rc, 0:272], p3h[0][:], AF.Abs)
                nc.scalar.activation(w1r_s[:, rc, 272:544], p3h[1][:], AF.Abs)

        def mix_prod(c0, cn, eng):
            w1v = w1r_s[:, c0:c0 + cn, :].rearrange(
                "p c (e v) -> p c e v", v=NV + 1)
            gqv = gq_s[:, c0:c0 + cn, :].rearrange(
                "p c (o v) -> p c o v", o=1)
            prodh = hpool.tile([128, cn, E, NV + 1], DT, tag="prodh")
            eng.tensor_tensor(prodh[:], w1v, bc(gqv, w1v), OP.mult)
            return prodh

        def mix_red(c0, cn, prodh):
            nc.vector.tensor_reduce(mix_s[:, c0:c0 + cn, :], prodh[:],
                                    AX.X, OP.add)

        def mix_chunk(c0, cn, eng):
            mix_red(c0, cn, mix_prod(c0, cn, eng))

        def mix_pair(c0, eng0, eng1, split_red=False):
            # one shared product tile, halves on two engines. split_red
            # reduces each half separately so the first reduce can run
            # while the second product still waits on its w1r eviction.
            prodh = hpool.tile([128, 2, E, NV + 1], DT, tag="prodh")
            for i, eng in ((0, eng0), (1, eng1)):
                w1v = w1r_s[:, c0 + i:c0 + i + 1, :].rearrange(
                    "p c (e v) -> p c e v", v=NV + 1)
                gqv = gq_s[:, c0 + i:c0 + i + 1, :].rearrange(
                    "p c (o v) -> p c o v", o=1)
                eng.tensor_tensor(prodh[:, i:i + 1], w1v, bc(gqv, w1v),
                                  OP.mult)
                if split_red:
                    nc.vector.tensor_reduce(
                        mix_s[:, c0 + i:c0 + i + 1, :], prodh[:, i:i + 1],
                        AX.X, OP.add)
            if not split_red:
                nc.vector.tensor_reduce(mix_s[:, c0:c0 + 2, :], prodh[:],
                                        AX.X, OP.add)

        w2r_s = pool.tile([128, C, E], DT)

        def w2_block(rcps):
            for rcp in rcps:
                p5 = psum.tile([128, 2, E], F32, tag="s1r", bufs=2)
                nc.tensor.matmul(p5[:].rearrange("p a b -> p (a b)"),
                                 ones_s[:, 0:128], brow_w2x2_s,
                                 start=True, stop=False)
                for rl in range(2):
                    rc = rcp * 2 + rl
                    for kc in range(2):
                        nc.tensor.matmul(p5[:, rl, :],
                                         z_s[:, 4 + kc, rc * 128:(rc + 1) * 128],
                                         w2l2(kc), start=False, stop=(kc == 1))
                nc.scalar.activation(w2r_s[:, rcp * 2:rcp * 2 + 2, :],
                                     p5[:], AF.Abs)

        # ---- hidden = elu(mix + b1), y = sum_e (hid-1)*|w2| + b2 --------
        # wsub = w2sum - b2v;  y = (ysum + b2_l2_b) - wsub
        w2sum_s = pool.tile([128, C], F32)
        wsub_s = pool.tile([128, C], F32)
        hidp_s = pool.tile([128, C, E], F32)
        m_s = pool.tile([128, C, E], F32)
        e_s = pool.tile([128, C, E], F32)
        hid_s = pool.tile([128, C, E], F32)   # = elu(hidp) + 1
        prodf_s = pool.tile([128, C, E], F32)
        ysum_s = pool.tile([128, C], F32)
        y_s = pool.tile([128, C], F32)

        # hidden = elu(hidp) computed directly: e-1 after the EXP makes the
        # -sum(w2) correction term vanish, so no w2sum/wsub reduce at all.
        def final_block(c0, cn, hidp_eng=None):
            cs = slice(c0, c0 + cn)
            (hidp_eng or nc.vector).tensor_tensor(
                hidp_s[:, cs, :], mix_s[:, cs, :],
                small_s[:, cs, 12:44], OP.add)
            nc.vector.tensor_single_scalar(m_s[:, cs, :], hidp_s[:, cs, :],
                                           0.0, OP.min)
            nc.scalar.activation(e_s[:, cs, :], m_s[:, cs, :], AF.Exp)
            nc.vector.tensor_single_scalar(m_s[:, cs, :], e_s[:, cs, :],
                                           1.0, OP.subtract)
            nc.vector.scalar_tensor_tensor(hid_s[:, cs, :], hidp_s[:, cs, :],
                                           0.0, m_s[:, cs, :], OP.max, OP.add)
            nc.vector.tensor_tensor(prodf_s[:, cs, :], hid_s[:, cs, :],
                                    w2r_s[:, cs, :], OP.mult)
            nc.vector.tensor_reduce(ysum_s[:, cs], prodf_s[:, cs, :],
                                    AX.X, OP.add)
            nc.vector.scalar_tensor_tensor(y_s[:, cs], ysum_s[:, cs],
                                           consts_s[:, E + 1:E + 2],
                                           b2v_s[:, cs], OP.add, OP.add)
            nc.sync.dma_start(out_d[:, cs], y_s[:, cs])

        # ---- phase order: rh0 pass unlocks rc 0-3 of everything, rh1
        # unlocks rc 4-7; downstream halves pipeline behind the PE stream.
        ps0 = s1t_alloc(0)
        s1t_round(0, ps0, 0)
        s1t_round(0, ps0, 1)
        s1t_kc23(0, ps0)
        # small0 after the full rh0 pass: its preload needs the TAILA DMA
        # (brow rows), which lands ~13us — interleaving it into the rounds
        # head-of-line blocks the PE FIFO on that DMA.
        p2a = small_preload(0)
        for _kc in range(4):
            small_kc(0, p2a, _kc)
        w1_block((0, 1))
        w0c_block((0, 1))       # sparse (N=4) — sandwiched between the
        w0d_block()             # dense w1 trains to keep HAM duty up
        w1_block((2, 3))
        b2_head(0)
        group_half(0)
        ph01 = mix_prod(0, 2, nc.gpsimd)
        p2b = small_preload(1)
        for _kc in range(4):
            small_kc(1, p2b, _kc)   # kc-grouped: fires as pair-b halves land
        ps1 = s1t_alloc(1)
        s1t_round(1, ps1, 0)
        s1t_round(1, ps1, 1)
        mix_chunk(2, 2, nc.vector)
        mix_red(0, 2, ph01)
        s1t_kc23(1, ps1)
        w0c_block((2, 3))
        b2_head(1)
        group_half(1)
        w1_block((4,))
        w1_block((5,))
        mix_pair(4, nc.vector, nc.vector)
        w2_block((0, 1))
        final_block(0, 4)       # runs on DVE while the PE does w1(6),(7)
        w1_block((6,))
        w2_block((2,))
        final_block(4, 2)
        w1_block((7,))
        mix_pair(6, nc.gpsimd, nc.vector, split_red=True)
        w2_block((3,))
        final_block(6, 2)

    nc.compile()
    return nc


def _prep_inputs(inputs):
    g = lambda k: np.asarray(inputs[k], dtype=np.float32)
    states = g('states')
    qvals = g('qvals')
    cr = np.asarray(inputs['causal_relations'])

    w00_l1_W, w00_l1_b = g('w00_l1_W'), g('w00_l1_b')
    b00_W, b00_b = g('b00_W'), g('b00_b')
    h_delta = w00_l1_W[SD:].sum(0)
    g_delta = float(b00_W[SD:].sum(0)[0])

    wcat = np.concatenate([w00_l1_W[:SD], g('w1_l1_W'), g('w2_l1_W')], axis=1)
    b_cat = np.concatenate([w00_l1_b, g('w1_l1_b'), g('w2_l1_b')])
    bias_t = np.zeros((128, 8), np.float32)
    for fc in range(6):
        bias_t[:, fc] = b_cat[fc * 128:(fc + 1) * 128]
    for fc in range(2):
        bias_t[:, 6 + fc] = (w00_l1_b + h_delta)[fc * 128:(fc + 1) * 128]

    wsmall = np.concatenate([g('w01_W'), g('b01_W'), b00_W[:SD],
                             g('b1_W'), g('b2_l1_W')], axis=1)
    brow_small = np.concatenate([g('w01_b'), g('b01_b'), b00_b,
                                 g('b1_b'), g('b2_l1_b')])
    perm = np.array([v * E + e for e in range(E) for v in range(NV + 1)])
    w1l2 = g('w1_l2_W')[:, perm]
    brow_w1 = g('w1_l2_b')[perm]
    w2l2, brow_w2 = g('w2_l2_W'), g('w2_l2_b')
    w0l2, brow_w0 = g('w00_l2_W'), g('w00_l2_b')

    # shared bf16 mega columns (everything except s_t / gat / qvb)
    mb_shared = np.zeros((128, NBF), BF)
    for kc in range(4):
        mb_shared[:, OFF_PAIR + kc * PAIR:OFF_PAIR + kc * PAIR + 768] = \
            wcat[kc * 128:(kc + 1) * 128]
        mb_shared[:, OFF_WSMALL + kc * SMALL:OFF_WSMALL + (kc + 1) * SMALL] = \
            wsmall[kc * 128:(kc + 1) * 128]
    for kc in range(2):
        mb_shared[:, OFF_W1L2 + kc * W1COL:OFF_W1L2 + (kc + 1) * W1COL] = \
            w1l2[kc * 128:(kc + 1) * 128]
        mb_shared[:, OFF_W2L2 + kc * E:OFF_W2L2 + (kc + 1) * E] = \
            w2l2[kc * 128:(kc + 1) * 128]
        mb_shared[:, OFF_W0L2 + kc * K:OFF_W0L2 + (kc + 1) * K] = \
            w0l2[kc * 128:(kc + 1) * 128]
    o = OFF_BROW
    mb_shared[0, o:o + 4 * SMALL] = np.tile(brow_small, 4)
    mb_shared[0, o + 4 * SMALL:o + 4 * SMALL + W1COL] = brow_w1
    mb_shared[0, o + 4 * SMALL + W1COL:o + 4 * SMALL + W1COL + E + K] = \
        np.concatenate([brow_w2, brow_w0])
    mb_shared[0, OFF_BROW2:OFF_BROW2 + 2 * E] = np.tile(brow_w2, 2)
    mb_shared[0, OFF_BROW2 + 2 * E:OFF_BROW2 + 2 * E + 2 * K] = \
        np.tile(brow_w0, 2)

    mf_shared = np.zeros((128, NF32), np.float32)
    mf_shared[:, GOFF_BIAS:GOFF_BIAS + 8] = bias_t
    mf_shared[:, GOFF_CONSTS:GOFF_CONSTS + E] = g('b2_l2_W')[:, 0][None, :]
    mf_shared[:, GOFF_CONSTS + E] = g_delta
    mf_shared[:, GOFF_CONSTS + E + 1] = float(g('b2_l2_b')[0])
    mf_shared[:, GOFF_BIASN:GOFF_BIASN + 8] = -bias_t

    to_pc = lambda x: np.ascontiguousarray(
        x.reshape(C, 128, -1).transpose(1, 0, 2).reshape(128, -1))

    in_maps = []
    for m in range(NCORES):
        bs = m + 8 * np.arange(16)
        mb = mb_shared.copy()
        S2 = states[bs].reshape(R, SD)
        s_tT = np.ascontiguousarray(S2.T).astype(BF)    # [512, R]
        for kc in range(4):
            mb[:, OFF_PAIR + kc * PAIR + 768:OFF_PAIR + (kc + 1) * PAIR] = \
                s_tT[kc * 128:(kc + 1) * 128]

        qv = qvals[bs].reshape(R, NA)
        cr_vk = np.swapaxes(cr[bs].reshape(R, K, NV), 1, 2)  # [r, v, k]
        gat = np.take_along_axis(
            np.broadcast_to(qv[:, None, :], (R, NV, NA)), cr_vk, axis=-1)
        crx = np.zeros((R, JW), np.float32)
        crx[:, 0:NVK] = gat.reshape(R, NVK)
        vd = np.where(np.arange(128) < 64, m, m + 8)
        crx[0:128, NVK:JW] = gat[np.arange(128), vd, :]
        mb[:, OFF_CRX:OFF_CRX + C * JW] = to_pc(crx)
        mb[:, OFF_QVB:OFF_QVB + C * NA] = to_pc(qv)

        mf = mf_shared.copy()
        dmask = np.zeros((128, NV), np.float32)
        dmask[np.arange(128), vd] = 1.0
        mf[:, GOFF_DMASK:GOFF_DMASK + NV] = dmask
        in_maps.append(dict(mb=mb, mf=mf))
    return in_maps


def kernel(**inputs):
    if 'nc' not in _cache:
        _cache['nc'] = _build_nc()
    nc = _cache['nc']
    in_maps = _prep_inputs(inputs)
    res = run_bass_kernel_spmd(nc, in_maps, list(range(NCORES)),
                               **_cache.get('run_kwargs', {}))
    _cache['last_result'] = res
    y = np.zeros((B, T, 1), np.float32)
    for m in range(NCORES):
        bs = m + 8 * np.arange(16)
        o = res.results[m]['out']               # [128, C]
        rows = np.ascontiguousarray(o.T).reshape(R)   # r = c*128+p
        y[bs] = rows.reshape(16, T, 1)
    return y



# revision 54
# speedup vs baseline: 1.0121x; 1.0121x over previous
"""Trainium2 Bass kernel for nn_CausalMixer (QMIX-style causal mixer).

Data-parallel across 8 NeuronCores: batch dim sharded round-robin
(core m gets batches m, m+8, m+16, ...), hypernet weights replicated.

Per-core layout (R = 1024 rows = 16 batches x 64 timesteps):
  - stage-1 "transposed" GEMMs: out[feat, rows] = Wcat.T-chunks @ states.T,
    evicted with fused per-partition bias+ReLU (alternating ScalarE / DVE).
  - stage-2 row-major GEMMs: the relu'd z tiles [feat, rows] serve directly
    as lhsT, producing per-row hypernet weights [rows, feat]; bias is
    preloaded into PSUM with a K=1 ones-matmul.
  - gather (qvals[cr]) precomputed on host, shipped in the mega tensor.
  - the onehot quirk (batch row b==v gets +delta) is handled as a rank-1
    correction on chunk 0 only (host orders the diag batches first).
  - all bf16 inputs ride one mega-packed DRAM tensor split across the two
    HW DGE queues (SP + Activation), critical (wcat,states) pairs first.

Scheduling (tuned against perfetto traces):
  - a 10-matmul K=128/N=512 warmup train on a memset tile runs from kernel
    start so the HAM clock gate flips to 8/8 (~2.4 GHz) before the real
    GEMMs; K=1 ones-matmuls do NOT register as array activity.
  - the s1t passes run kc-major across all 6 feature chunks (6 concurrent
    PSUM banks), so each arriving (wcat|s_t) DMA pair unlocks 6 matmuls
    and the PE FIFO never head-of-line blocks on a late pair; kc 2,3 then
    go fc-by-fc (z1 first) so evictions start early.
  - everything downstream is split into batch-halves (rc 0-3 / 4-7) and
    pipelined behind the PE stream; sparse N=4/N=32 blocks are sandwiched
    between dense w1 trains to keep HAM array-duty up; the final
    elu/y blocks are consolidated (0..5 and 6..7) with per-half output
    DMAs so only the last two chunks gate the kernel end.
"""

import sys

for _p in ("/root/.axon_site", "/root/.axon_site/_ro/trn_rl_repo",
           "/root/.axon_site/_ro/pypackages", "/opt/trn_rl_repo"):
    if _p not in sys.path:
        sys.path.append(_p)

import numpy as np
import ml_dtypes
from contextlib import ExitStack

import concourse.bass as bass
import concourse.bacc as bacc_mod
import concourse.tile as tile
import concourse.mybir as mybir
from concourse.bass import broadcast_tensor_aps
from concourse.bass_utils import run_bass_kernel_spmd

BF = ml_dtypes.bfloat16
DT = mybir.dt.bfloat16
F32 = mybir.dt.float32
OP = mybir.AluOpType
AF = mybir.ActivationFunctionType
AX = mybir.AxisListType

NCORES = 8
B, T, NA, NV, K, SD, H, E = 128, 64, 10, 16, 4, 512, 256, 32
R = 16 * T            # rows per core = 1024
C = R // 128          # row chunks per core = 8
NVK = NV * K          # 64
JW = NVK + K          # 68 gather cols (64 + 4 diag)
SMALL = NA + 1 + 1 + E + E   # 76: w01 | b01 | b00 | b1 | b2_l1
W1COL = (NV + 1) * E  # 544

# ---- mega-packed bf16 input column map (need-ordered) -------------------
# gat 8x68 | qvb 8x10, then 4x [wcat_kc (768) | s_t_kc (1024)] pairs, then:
#   wsmall 4x76 | w1l2 2x544 | w2l2 2x32 | w0l2 2x4
#   | brow_small4 4x76 | brow_w1 544 | brow_w20 36 | brow_w2x2 64
#   | brow_w0x2 8  (partition 0)
PAIR = 768 + R
OFF_CRX = 0
OFF_QVB = OFF_CRX + C * JW
OFF_PAIR = OFF_QVB + C * NA
OFF_TAILA = OFF_PAIR + 4 * PAIR
OFF_WSMALL = OFF_TAILA
OFF_W0L2 = OFF_WSMALL + 4 * SMALL
OFF_BROW = OFF_W0L2 + 2 * K
OFF_BROW2 = OFF_BROW + 4 * SMALL + W1COL + E + K
OFF_TAILB = OFF_BROW2 + 2 * E + 2 * K
OFF_W1L2 = OFF_TAILB
OFF_W2L2 = OFF_W1L2 + 2 * W1COL
NBF = OFF_W2L2 + 2 * E
# f32 mega: bias_t (8) | dmask (16) | consts (34)
GOFF_BIAS = 0
GOFF_DMASK = 8
GOFF_CONSTS = 24
GOFF_BIASN = GOFF_CONSTS + E + 2
NF32 = GOFF_BIASN + 8

_cache = {}


def _build_nc():
    nc = bacc_mod.Bacc("TRN2", target_bir_lowering=False, debug=False)

    mb_d = nc.dram_tensor("mb", [128, NBF], DT, kind="ExternalInput")
    mf_d = nc.dram_tensor("mf", [128, NF32], F32, kind="ExternalInput")
    out_d = nc.dram_tensor("out", [128, C], F32, kind="ExternalOutput")

    with tile.TileContext(nc) as tc, ExitStack() as ctx:
        pool = ctx.enter_context(tc.tile_pool(name="sbuf", bufs=1))
        hpool = ctx.enter_context(tc.tile_pool(name="hbuf", bufs=3))
        psum = ctx.enter_context(tc.tile_pool(name="psum", bufs=2, space="PSUM"))

        mb_s = pool.tile([128, NBF], DT)
        mf_s = pool.tile([128, NF32], F32)

        # ---- DMAs on both HW DGE queues; (wcat | s_t rh0) pairs first ----
        def dma_cols(eng, c0, c1):
            eng.dma_start(mb_s[:, c0:c1], mb_d[:, c0:c1])

        nc.scalar.dma_start(mf_s[:], mf_d[:])
        dma_cols(nc.sync, OFF_PAIR + 0 * PAIR, OFF_PAIR + 0 * PAIR + 1280)
        dma_cols(nc.scalar, OFF_PAIR + 1 * PAIR, OFF_PAIR + 1 * PAIR + 1280)
        dma_cols(nc.sync, OFF_PAIR + 2 * PAIR, OFF_PAIR + 2 * PAIR + 1280)
        dma_cols(nc.scalar, OFF_PAIR + 3 * PAIR, OFF_PAIR + 3 * PAIR + 1280)
        dma_cols(nc.sync, OFF_TAILB, NBF)         # w1l2 | w2l2 (w1 blocks)
        dma_cols(nc.scalar, OFF_TAILA, OFF_TAILB)  # wsmall | w0l2 | bias rows
        dma_cols(nc.sync, OFF_PAIR + 0 * PAIR + 1280, OFF_PAIR + 1 * PAIR)
        dma_cols(nc.scalar, OFF_PAIR + 1 * PAIR + 1280, OFF_PAIR + 2 * PAIR)
        dma_cols(nc.sync, OFF_PAIR + 2 * PAIR + 1280, OFF_PAIR + 3 * PAIR)
        dma_cols(nc.scalar, OFF_CRX, OFF_PAIR)     # gat | qvb (group chain)
        dma_cols(nc.scalar, OFF_PAIR + 3 * PAIR + 1280, OFF_PAIR + 4 * PAIR)

        def wcat(kc, c0, c1):
            return mb_s[:, OFF_PAIR + kc * PAIR + c0:OFF_PAIR + kc * PAIR + c1]

        def s_t(kc, c0, c1):
            return mb_s[:, OFF_PAIR + kc * PAIR + 768 + c0:
                        OFF_PAIR + kc * PAIR + 768 + c1]

        def wsmall(kc):
            return mb_s[:, OFF_WSMALL + kc * SMALL:OFF_WSMALL + (kc + 1) * SMALL]

        def w1l2(kc, c0, c1):
            return mb_s[:, OFF_W1L2 + kc * W1COL + c0:OFF_W1L2 + kc * W1COL + c1]

        def w2l2(kc):
            return mb_s[:, OFF_W2L2 + kc * E:OFF_W2L2 + (kc + 1) * E]

        def w0l2(kc):
            return mb_s[:, OFF_W0L2 + kc * K:OFF_W0L2 + (kc + 1) * K]

        acc_s = mb_s[:, OFF_CRX:OFF_CRX + C * JW].rearrange(
            "p (c j) -> p c j", j=JW)          # host-gathered q-values
        qvb_s = mb_s[:, OFF_QVB:OFF_QVB + C * NA].rearrange(
            "p (c j) -> p c j", j=NA)
        brow_small4_s = mb_s[0:1, OFF_BROW:OFF_BROW + 4 * SMALL]
        brow_w1_s = mb_s[0:1, OFF_BROW + 4 * SMALL:OFF_BROW + 4 * SMALL + W1COL]
        brow_w0_s = mb_s[0:1, OFF_BROW + 4 * SMALL + W1COL + E:
                         OFF_BROW + 4 * SMALL + W1COL + E + K]
        brow_w2x2_s = mb_s[0:1, OFF_BROW2:OFF_BROW2 + 2 * E]
        brow_w0x2_s = mb_s[0:1, OFF_BROW2 + 2 * E:OFF_BROW2 + 2 * E + 2 * K]
        bias_t_s = mf_s[:, GOFF_BIAS:GOFF_BIAS + 8]
        dmask_s = mf_s[:, GOFF_DMASK:GOFF_DMASK + NV]
        consts_s = mf_s[:, GOFF_CONSTS:GOFF_CONSTS + E + 2]
        biasn_s = mf_s[:, GOFF_BIASN:GOFF_BIASN + 8]

        # ---- PE warmup: HAM counts *array* activity, so K=1 ones-matmuls
        # do NOT lift the clock gate — use full K=128 matmuls on a memset
        # tile. The train bridges from kernel start until the first
        # wcat/s_t pair lands (~12us) so the real s1t GEMMs run at 2.4 GHz.
        # warm_s memset goes FIRST on gpsimd: it gates the train's start,
        # while ones_s isn't read until the small preload (~13us).
        def bc(ap, like):
            a, _ = broadcast_tensor_aps(ap, like)
            return a

        warm_s = pool.tile([128, 512], DT)
        nc.gpsimd.memset(warm_s[:], 0.0)

        ones_s = pool.tile([1, 512], DT)
        nc.gpsimd.memset(ones_s[:], 1.0)
        for i in range(10):
            pw = psum.tile([128, 512], F32, tag="s1r", bufs=2)
            nc.tensor.matmul(pw[:], warm_s[:, 0:128], warm_s[:],
                             start=True, stop=True)

        # ---- stage-1 transposed GEMMs: z = relu(Wcat.T @ states + b) ----
        z_s = pool.tile([128, 6, R], DT)      # zA | z1 | z2 (feat-major)
        zad_s = pool.tile([128, 2, 128], DT)  # diag zA, chunk 0 only

        def s1t_evict(p1, fc, rh):
            dst = z_s[:, fc, rh * 512:(rh + 1) * 512]
            # rh1: bias toward scalar (DVE only takes fc 3, 1) — the DVE
            # FIFO carries the mix chain for rc 0-3 in that window.
            on_dve = (fc % 2 == 0) if rh == 0 else (fc in (3, 1))
            if on_dve:
                nc.vector.scalar_tensor_tensor(
                    dst, p1[:], biasn_s[:, fc:fc + 1],
                    bc(bias_t_s[:, fc:fc + 1], p1[:]),
                    OP.max, OP.add)
            else:
                nc.scalar.activation(dst, p1[:], AF.Relu,
                                     bias=bias_t_s[:, fc:fc + 1])
            if rh == 0 and fc < 2:
                nc.scalar.activation(zad_s[:, fc, :], p1[:, 0:128],
                                     AF.Relu,
                                     bias=bias_t_s[:, 6 + fc:7 + fc])

        # kc-major over all 6 feature chunks: each arriving (wcat, s_t) DMA
        # pair unlocks 6 matmuls, so the PE FIFO never head-of-line blocks
        # on a late pair. kc 2,3 then go fc-by-fc so evictions start early,
        # prioritising z1 (fc 2,3 -> w1 blocks) then z00 (group chain).
        def s1t_alloc(rh):
            return [psum.tile([128, 512], F32, tag="s1t", bufs=6,
                              name=f"p1_{rh}_{fc}") for fc in range(6)]

        def s1t_round(rh, ps, kc):
            for fc in range(6):
                nc.tensor.matmul(
                    ps[fc][:], wcat(kc, fc * 128, (fc + 1) * 128),
                    s_t(kc, rh * 512, (rh + 1) * 512),
                    start=(kc == 0), stop=False)

        def s1t_kc23(rh, ps, extras=None):
            # extras: {fc: callable} — sparse N=4/8 blocks woven between the
            # dense N=512 groups so no HAM window drops below the duty gate.
            for fc in (2, 3, 0, 1, 4, 5):
                for kc in (2, 3):
                    nc.tensor.matmul(
                        ps[fc][:], wcat(kc, fc * 128, (fc + 1) * 128),
                        s_t(kc, rh * 512, (rh + 1) * 512),
                        start=False, stop=(kc == 3))
                s1t_evict(ps[fc], fc, rh)
                if extras and fc in extras:
                    extras[fc]()

        # ---- stage-1 row-major small heads, kc-grouped so the N=76 runs
        # interleave with the dense s1t rounds (each small-kc group needs
        # exactly the DMA pair the preceding s1t round consumed, and the
        # alternation keeps HAM array-duty above the re-throttle point).
        small_s = pool.tile([128, C, SMALL], F32)  # w01|b01|b00|b1|zb2(pre-relu)

        def small_preload(g):
            p2 = psum.tile([128, 4, SMALL], F32, tag="s1r", bufs=2)
            nc.tensor.matmul(p2[:].rearrange("p a b -> p (a b)"),
                             ones_s[:, 0:128], brow_small4_s,
                             start=True, stop=False)
            return p2

        def small_kc(g, p2, kc):
            for rl in range(4):
                rc = g * 4 + rl
                nc.tensor.matmul(p2[:, rl, :],
                                 s_t(kc, rc * 128, (rc + 1) * 128),
                                 wsmall(kc), start=False, stop=(kc == 3))
            if kc == 3:
                nc.scalar.copy(small_s[:, g * 4:(g + 1) * 4, :], p2[:])

        # ---- w0c / w0d (needs z00 only; unblocks the group chain) -------
        w0c_s = pool.tile([128, C, K], DT)
        w0d_s = pool.tile([128, K], DT)       # |w0_diag|, chunk 0

        def w0c_block(rcps):
            for rcp in rcps:
                p4 = psum.tile([128, 2, K], F32, tag="s1r", bufs=2)
                nc.tensor.matmul(p4[:].rearrange("p a b -> p (a b)"),
                                 ones_s[:, 0:128], brow_w0x2_s,
                                 start=True, stop=False)
                for rl in range(2):
                    rc = rcp * 2 + rl
                    for kc in range(2):
                        nc.tensor.matmul(p4[:, rl, :],
                                         z_s[:, 0 + kc, rc * 128:(rc + 1) * 128],
                                         w0l2(kc), start=False, stop=(kc == 1))
                nc.scalar.activation(w0c_s[:, rcp * 2:rcp * 2 + 2, :],
                                     p4[:], AF.Abs)

        def w0d_block():
            p6 = psum.tile([128, K], F32, tag="s1r", bufs=2)
            nc.tensor.matmul(p6[:], ones_s[:, 0:128], brow_w0_s,
                             start=True, stop=False)
            for kc in range(2):
                nc.tensor.matmul(p6[:], zad_s[:, kc, :], w0l2(kc),
                                 start=False, stop=(kc == 1))
            nc.scalar.activation(w0d_s[:], p6[:], AF.Abs)

        # ---- b2 head (needs only small_s / consts), per batch-half ------
        zb2r_s = pool.tile([128, C, E], F32)
        b2p_s = pool.tile([128, C, E], F32)
        b2v_s = pool.tile([128, C], F32)
        cb2 = consts_s[:, 0:E].rearrange("p (o e) -> p o e", o=1)

        def b2_head(g):
            cs = slice(g * 4, (g + 1) * 4)
            nc.scalar.activation(zb2r_s[:, cs, :], small_s[:, cs, 44:76],
                                 AF.Relu)
            nc.gpsimd.tensor_tensor(b2p_s[:, cs, :], zb2r_s[:, cs, :],
                                    bc(cb2, zb2r_s[:, cs, :]), OP.mult)
            nc.vector.tensor_reduce(b2v_s[:, cs], b2p_s[:, cs, :],
                                    AX.X, OP.add)

        # ---- group values + "other" head + gq assembly, per half --------
        gath4 = acc_s[:, :, 0:NVK].rearrange("p c (v k) -> p c v k", k=K)
        w04 = w0c_s.rearrange("p c (o k) -> p c o k", o=1)
        prodg_s = pool.tile([128, C, NV, K], DT)
        group_s = pool.tile([128, C, NV], F32)
        prodo_s = pool.tile([128, C, NA], F32)
        other_s = pool.tile([128, C], F32)
        gq_s = pool.tile([128, C, NV + 1], DT)

        def group_half(g):
            cs = slice(g * 4, (g + 1) * 4)
            nc.vector.tensor_tensor(prodg_s[:, cs], gath4[:, cs],
                                    bc(w04[:, cs], gath4[:, cs]), OP.mult)
            nc.vector.tensor_reduce(group_s[:, cs], prodg_s[:, cs],
                                    AX.X, OP.add)
            gb = small_s[:, cs, 11:12]
            nc.vector.tensor_tensor(group_s[:, cs], group_s[:, cs],
                                    bc(gb, group_s[:, cs]), OP.add)
            if g == 0:
                # diag correction (chunk 0 only)
                dw_s = pool.tile([128, K], F32)
                nc.vector.tensor_tensor(dw_s[:], w0d_s[:], w0c_s[:, 0, :],
                                        OP.subtract)
                gselp_s = pool.tile([128, K], F32)
                nc.vector.tensor_tensor(gselp_s[:], acc_s[:, 0, NVK:JW],
                                        dw_s[:], OP.mult)
                corr0_s = pool.tile([128, 1], F32)
                nc.vector.tensor_reduce(corr0_s[:], gselp_s[:], AX.X, OP.add)
                corr_s = pool.tile([128, 1], F32)
                nc.vector.tensor_scalar(corr_s[:], corr0_s[:],
                                        consts_s[:, E:E + 1], None, OP.add)
                nc.vector.scalar_tensor_tensor(group_s[:, 0, :], dmask_s,
                                               corr_s[:], group_s[:, 0, :],
                                               OP.mult, OP.add)
            nc.vector.tensor_tensor(prodo_s[:, cs], qvb_s[:, cs],
                                    small_s[:, cs, 0:NA], OP.mult)
            nc.vector.tensor_reduce(other_s[:, cs], prodo_s[:, cs],
                                    AX.X, OP.add)
            nc.vector.tensor_tensor(other_s[:, cs], other_s[:, cs],
                                    small_s[:, cs, NA], OP.add)
            nc.gpsimd.tensor_copy(gq_s[:, cs, 0:NV], group_s[:, cs])
            nc.gpsimd.tensor_copy(
                gq_s[:, cs, NV:NV + 1],
                other_s[:, cs].rearrange("p (c o) -> p c o", o=1))


        # ---- stage-2 w1 GEMMs + per-rc |w1| evictions + gq mix ----------
        # w1r cols are e-major (col = e*17+v); mix[p,rc,e] = sum_v gq*|w1|
        w1r_s = pool.tile([128, C, W1COL], DT)
        mix_s = pool.tile([128, C, E], F32)

        def w1_block(rcs):
            for rc in rcs:
                p3h = []
                for h in range(2):
                    p3 = psum.tile([128, 272], F32, tag="s1t", bufs=6, name="p3")
                    p3h.append(p3)
                    nc.tensor.matmul(p3[:], ones_s[:, 0:128],
                                     brow_w1_s[:, h * 272:(h + 1) * 272],
                                     start=True, stop=False)
                for kc in range(2):
                    for h in range(2):
                        nc.tensor.matmul(p3h[h][:],
                                         z_s[:, 2 + kc, rc * 128:(rc + 1) * 128],
                                         w1l2(kc, h * 272, (h + 1) * 272),
                                         start=False, stop=(kc == 1))
                nc.scalar.activation(w1r_s[:, rc, 0:272], p3h[0][:], AF.Abs)
                nc.scalar.activation(w1r_s[:, rc, 272:544], p3h[1][:], AF.Abs)

        def mix_prod(c0, cn, eng):
            w1v = w1r_s[:, c0:c0 + cn, :].rearrange(
                "p c (e v) -> p c e v", v=NV + 1)
            gqv = gq_s[:, c0:c0 + cn, :].rearrange(
                "p c (o v) -> p c o v", o=1)
            prodh = hpool.tile([128, cn, E, NV + 1], DT, tag="prodh")
            eng.tensor_tensor(prodh[:], w1v, bc(gqv, w1v), OP.mult)
            return prodh

        def mix_red(c0, cn, prodh):
            nc.vector.tensor_reduce(mix_s[:, c0:c0 + cn, :], prodh[:],
                                    AX.X, OP.add)

        def mix_chunk(c0, cn, eng):
            mix_red(c0, cn, mix_prod(c0, cn, eng))

        def mix_pair(c0, eng0, eng1, split_red=False):
            # one shared product tile, halves on two engines. split_red
            # reduces each half separately so the first reduce can run
            # while the second product still waits on its w1r eviction.
            prodh = hpool.tile([128, 2, E, NV + 1], DT, tag="prodh")
            for i, eng in ((0, eng0), (1, eng1)):
                w1v = w1r_s[:, c0 + i:c0 + i + 1, :].rearrange(
                    "p c (e v) -> p c e v", v=NV + 1)
                gqv = gq_s[:, c0 + i:c0 + i + 1, :].rearrange(
                    "p c (o v) -> p c o v", o=1)
                eng.tensor_tensor(prodh[:, i:i + 1], w1v, bc(gqv, w1v),
                                  OP.mult)
                if split_red:
                    nc.vector.tensor_reduce(
                        mix_s[:, c0 + i:c0 + i + 1, :], prodh[:, i:i + 1],
                        AX.X, OP.add)
            if not split_red:
                nc.vector.tensor_reduce(mix_s[:, c0:c0 + 2, :], prodh[:],
                                        AX.X, OP.add)

        w2r_s = pool.tile([128, C, E], DT)

        def w2_block(rcps):
            for rcp in rcps:
                p5 = psum.tile([128, 2, E], F32, tag="s1r", bufs=2)
                nc.tensor.matmul(p5[:].rearrange("p a b -> p (a b)"),
                                 ones_s[:, 0:128], brow_w2x2_s,
                                 start=True, stop=False)
                for rl in range(2):
                    rc = rcp * 2 + rl
                    for kc in range(2):
                        nc.tensor.matmul(p5[:, rl, :],
                                         z_s[:, 4 + kc, rc * 128:(rc + 1) * 128],
                                         w2l2(kc), start=False, stop=(kc == 1))
                nc.scalar.activation(w2r_s[:, rcp * 2:rcp * 2 + 2, :],
                                     p5[:], AF.Abs)

        # ---- hidden = elu(mix + b1), y = sum_e (hid-1)*|w2| + b2 --------
        # wsub = w2sum - b2v;  y = (ysum + b2_l2_b) - wsub
        w2sum_s = pool.tile([128, C], F32)
        wsub_s = pool.tile([128, C], F32)
        hidp_s = pool.tile([128, C, E], F32)
        m_s = pool.tile([128, C, E], F32)
        e_s = pool.tile([128, C, E], F32)
        hid_s = pool.tile([128, C, E], F32)   # = elu(hidp) + 1
        prodf_s = pool.tile([128, C, E], F32)
        ysum_s = pool.tile([128, C], F32)
        y_s = pool.tile([128, C], F32)

        # hidden = elu(hidp) computed directly: e-1 after the EXP makes the
        # -sum(w2) correction term vanish, so no w2sum/wsub reduce at all.
        def final_block(c0, cn, hidp_eng=None):
            cs = slice(c0, c0 + cn)
            (hidp_eng or nc.vector).tensor_tensor(
                hidp_s[:, cs, :], mix_s[:, cs, :],
                small_s[:, cs, 12:44], OP.add)
            nc.vector.tensor_single_scalar(m_s[:, cs, :], hidp_s[:, cs, :],
                                           0.0, OP.min)
            nc.scalar.activation(e_s[:, cs, :], m_s[:, cs, :], AF.Exp)
            nc.vector.tensor_single_scalar(m_s[:, cs, :], e_s[:, cs, :],
                                           1.0, OP.subtract)
            nc.vector.scalar_tensor_tensor(hid_s[:, cs, :], hidp_s[:, cs, :],
                                           0.0, m_s[:, cs, :], OP.max, OP.add)
            nc.vector.tensor_tensor(prodf_s[:, cs, :], hid_s[:, cs, :],
                                    w2r_s[:, cs, :], OP.mult)
            nc.vector.tensor_reduce(ysum_s[:, cs], prodf_s[:, cs, :],
                                    AX.X, OP.add)
            nc.vector.scalar_tensor_tensor(y_s[:, cs], ysum_s[:, cs],
                                           consts_s[:, E + 1:E + 2],
                                           b2v_s[:, cs], OP.add, OP.add)
            nc.sync.dma_start(out_d[:, cs], y_s[:, cs])

        # ---- phase order: rh0 pass unlocks rc 0-3 of everything, rh1
        # unlocks rc 4-7; downstream halves pipeline behind the PE stream.
        ps0 = s1t_alloc(0)
        s1t_round(0, ps0, 0)
        s1t_round(0, ps0, 1)
        s1t_kc23(0, ps0)
        # small0 after the full rh0 pass: its preload needs the TAILA DMA
        # (brow rows), which lands ~13us — interleaving it into the rounds
        # head-of-line blocks the PE FIFO on that DMA.
        p2a = small_preload(0)
        for _kc in range(4):
            small_kc(0, p2a, _kc)
        w1_block((0, 1))
        w0c_block((0, 1))       # sparse (N=4) — sandwiched between the
        w0d_block()             # dense w1 trains to keep HAM duty up
        w1_block((2, 3))
        b2_head(0)
        group_half(0)
        ph01 = mix_prod(0, 2, nc.gpsimd)
        p2b = small_preload(1)
        for _kc in range(4):
            small_kc(1, p2b, _kc)   # kc-grouped: fires as pair-b halves land
        ps1 = s1t_alloc(1)
        s1t_round(1, ps1, 0)
        s1t_round(1, ps1, 1)
        mix_chunk(2, 2, nc.vector)
        mix_red(0, 2, ph01)
        s1t_kc23(1, ps1)
        w0c_block((2, 3))
        b2_head(1)
        group_half(1)
        w1_block((4,))
        w1_block((5,))
        mix_pair(4, nc.vector, nc.vector)
        w2_block((0, 1))
        final_block(0, 4)       # runs on DVE while the PE does w1(6),(7)
        w1_block((6,))
        w2_block((2,))
        final_block(4, 2)
        w1_block((7,))
        mix_pair(6, nc.gpsimd, nc.vector, split_red=True)
        w2_block((3,))
        final_block(6, 2)

    nc.compile()
    return nc


def _prep_inputs(inputs):
    g = lambda k: np.asarray(inputs[k], dtype=np.float32)
    states = g('states')
    qvals = g('qvals')
    cr = np.asarray(inputs['causal_relations'])

    w00_l1_W, w00_l1_b = g('w00_l1_W'), g('w00_l1_b')
    b00_W, b00_b = g('b00_W'), g('b00_b')
    h_delta = w00_l1_W[SD:].sum(0)
    g_delta = float(b00_W[SD:].sum(0)[0])

    wcat = np.concatenate([w00_l1_W[:SD], g('w1_l1_W'), g('w2_l1_W')], axis=1)
    b_cat = np.concatenate([w00_l1_b, g('w1_l1_b'), g('w2_l1_b')])
    bias_t = np.zeros((128, 8), np.float32)
    for fc in range(6):
        bias_t[:, fc] = b_cat[fc * 128:(fc + 1) * 128]
    for fc in range(2):
        bias_t[:, 6 + fc] = (w00_l1_b + h_delta)[fc * 128:(fc + 1) * 128]

    wsmall = np.concatenate([g('w01_W'), g('b01_W'), b00_W[:SD],
                             g('b1_W'), g('b2_l1_W')], axis=1)
    brow_small = np.concatenate([g('w01_b'), g('b01_b'), b00_b,
                                 g('b1_b'), g('b2_l1_b')])
    perm = np.array([v * E + e for e in range(E) for v in range(NV + 1)])
    w1l2 = g('w1_l2_W')[:, perm]
    brow_w1 = g('w1_l2_b')[perm]
    w2l2, brow_w2 = g('w2_l2_W'), g('w2_l2_b')
    w0l2, brow_w0 = g('w00_l2_W'), g('w00_l2_b')

    # shared bf16 mega columns (everything except s_t / gat / qvb)
    mb_shared = np.zeros((128, NBF), BF)
    for kc in range(4):
        mb_shared[:, OFF_PAIR + kc * PAIR:OFF_PAIR + kc * PAIR + 768] = \
            wcat[kc * 128:(kc + 1) * 128]
        mb_shared[:, OFF_WSMALL + kc * SMALL:OFF_WSMALL + (kc + 1) * SMALL] = \
            wsmall[kc * 128:(kc + 1) * 128]
    for kc in range(2):
        mb_shared[:, OFF_W1L2 + kc * W1COL:OFF_W1L2 + (kc + 1) * W1COL] = \
            w1l2[kc * 128:(kc + 1) * 128]
        mb_shared[:, OFF_W2L2 + kc * E:OFF_W2L2 + (kc + 1) * E] = \
            w2l2[kc * 128:(kc + 1) * 128]
        mb_shared[:, OFF_W0L2 + kc * K:OFF_W0L2 + (kc + 1) * K] = \
            w0l2[kc * 128:(kc + 1) * 128]
    o = OFF_BROW
    mb_shared[0, o:o + 4 * SMALL] = np.tile(brow_small, 4)
    mb_shared[0, o + 4 * SMALL:o + 4 * SMALL + W1COL] = brow_w1
    mb_shared[0, o + 4 * SMALL + W1COL:o + 4 * SMALL + W1COL + E + K] = \
        np.concatenate([brow_w2, brow_w0])
    mb_shared[0, OFF_BROW2:OFF_BROW2 + 2 * E] = np.tile(brow_w2, 2)
    mb_shared[0, OFF_BROW2 + 2 * E:OFF_BROW2 + 2 * E + 2 * K] = \
        np.tile(brow_w0, 2)

    mf_shared = np.zeros((128, NF32), np.float32)
    mf_shared[:, GOFF_BIAS:GOFF_BIAS + 8] = bias_t
    mf_shared[:, GOFF_CONSTS:GOFF_CONSTS + E] = g('b2_l2_W')[:, 0][None, :]
    mf_shared[:, GOFF_CONSTS + E] = g_delta
    mf_shared[:, GOFF_CONSTS + E + 1] = float(g('b2_l2_b')[0])
    mf_shared[:, GOFF_BIASN:GOFF_BIASN + 8] = -bias_t

    to_pc = lambda x: np.ascontiguousarray(
        x.reshape(C, 128, -1).transpose(1, 0, 2).reshape(128, -1))

    in_maps = []
    for m in range(NCORES):
        bs = m + 8 * np.arange(16)
        mb = mb_shared.copy()
        S2 = states[bs].reshape(R, SD)
        s_tT = np.ascontiguousarray(S2.T).astype(BF)    # [512, R]
        for kc in range(4):
            mb[:, OFF_PAIR + kc * PAIR + 768:OFF_PAIR + (kc + 1) * PAIR] = \
                s_tT[kc * 128:(kc + 1) * 128]

        qv = qvals[bs].reshape(R, NA)
        cr_vk = np.swapaxes(cr[bs].reshape(R, K, NV), 1, 2)  # [r, v, k]
        gat = np.take_along_axis(
            np.broadcast_to(qv[:, None, :], (R, NV, NA)), cr_vk, axis=-1)
        crx = np.zeros((R, JW), np.float32)
        crx[:, 0:NVK] = gat.reshape(R, NVK)
        vd = np.where(np.arange(128) < 64, m, m + 8)
        crx[0:128, NVK:JW] = gat[np.arange(128), vd, :]
        mb[:, OFF_CRX:OFF_CRX + C * JW] = to_pc(crx)
        mb[:, OFF_QVB:OFF_QVB + C * NA] = to_pc(qv)

        mf = mf_shared.copy()
        dmask = np.zeros((128, NV), np.float32)
        dmask[np.arange(128), vd] = 1.0
        mf[:, GOFF_DMASK:GOFF_DMASK + NV] = dmask
        in_maps.append(dict(mb=mb, mf=mf))
    return in_maps


def kernel(**inputs):
    if 'nc' not in _cache:
        _cache['nc'] = _build_nc()
    nc = _cache['nc']
    in_maps = _prep_inputs(inputs)
    res = run_bass_kernel_spmd(nc, in_maps, list(range(NCORES)),
                               **_cache.get('run_kwargs', {}))
    _cache['last_result'] = res
    y = np.zeros((B, T, 1), np.float32)
    for m in range(NCORES):
        bs = m + 8 * np.arange(16)
        o = res.results[m]['out']               # [128, C]
        rows = np.ascontiguousarray(o.T).reshape(R)   # r = c*128+p
        y[bs] = rows.reshape(16, T, 1)
    return y



# revision 55
# speedup vs baseline: 1.1987x; 1.1844x over previous
"""Trainium2 Bass kernel for nn_CausalMixer (QMIX-style causal mixer).

Data-parallel across 8 NeuronCores: batch dim sharded round-robin
(core m gets batches m, m+8, m+16, ...), hypernet weights replicated.

Per-core layout (R = 1024 rows = 16 batches x 64 timesteps):
  - stage-1 "transposed" GEMMs: out[feat, rows] = Wcat.T-chunks @ states.T,
    evicted with fused per-partition bias+ReLU (alternating ScalarE / DVE).
  - stage-2 row-major GEMMs: the relu'd z tiles [feat, rows] serve directly
    as lhsT, producing per-row hypernet weights [rows, feat]; bias is
    preloaded into PSUM with a K=1 ones-matmul.
  - gather (qvals[cr]) precomputed on host, shipped in the mega tensor.
  - the onehot quirk (batch row b==v gets +delta) is handled as a rank-1
    correction on chunk 0 only (host orders the diag batches first).
  - all bf16 inputs ride one mega-packed DRAM tensor split across the two
    HW DGE queues (SP + Activation), critical (wcat,states) pairs first.

Scheduling (tuned against perfetto traces):
  - a 10-matmul K=128/N=512 warmup train on a memset tile runs from kernel
    start so the HAM clock gate flips to 8/8 (~2.4 GHz) before the real
    GEMMs; K=1 ones-matmuls do NOT register as array activity.
  - the s1t passes run kc-major across all 6 feature chunks (6 concurrent
    PSUM banks), so each arriving (wcat|s_t) DMA pair unlocks 6 matmuls
    and the PE FIFO never head-of-line blocks on a late pair; kc 2,3 then
    go fc-by-fc (z1 first) so evictions start early.
  - everything downstream is split into batch-halves (rc 0-3 / 4-7) and
    pipelined behind the PE stream; sparse N=4/N=32 blocks are sandwiched
    between dense w1 trains to keep HAM array-duty up; the w2 blocks and
    final elu/y blocks (rc 0-3 / 4-5 / 6-7) are threaded through the last
    w1 blocks so only the rc 6-7 chain gates the kernel end, with the
    last paired mix reduce split so red6 overlaps prod7's eviction wait.
"""

import sys

for _p in ("/root/.axon_site", "/root/.axon_site/_ro/trn_rl_repo",
           "/root/.axon_site/_ro/pypackages", "/opt/trn_rl_repo"):
    if _p not in sys.path:
        sys.path.append(_p)

import numpy as np
import ml_dtypes
from contextlib import ExitStack

import concourse.bass as bass
import concourse.bacc as bacc_mod
import concourse.tile as tile
import concourse.mybir as mybir
from concourse.bass import broadcast_tensor_aps
from concourse.bass_utils import run_bass_kernel_spmd

BF = ml_dtypes.bfloat16
DT = mybir.dt.bfloat16
F32 = mybir.dt.float32
OP = mybir.AluOpType
AF = mybir.ActivationFunctionType
AX = mybir.AxisListType

NCORES = 8
B, T, NA, NV, K, SD, H, E = 128, 64, 10, 16, 4, 512, 256, 32
R = 16 * T            # rows per core = 1024
C = R // 128          # row chunks per core = 8
NVK = NV * K          # 64
JW = NVK + K          # 68 gather cols (64 + 4 diag)
SMALL = NA + 1 + 1 + E + E   # 76: w01 | b01 | b00 | b1 | b2_l1
W1COL = (NV + 1) * E  # 544

# ---- mega-packed bf16 input column map (need-ordered) -------------------
# gat 8x68 | qvb 8x10, then 4x [wcat_kc (768) | s_t_kc (1024)] pairs, then:
#   wsmall 4x76 | w1l2 2x544 | w2l2 2x32 | w0l2 2x4
#   | brow_small4 4x76 | brow_w1 544 | brow_w20 36 | brow_w2x2 64
#   | brow_w0x2 8  (partition 0)
PAIR = 768 + R
OFF_CRX = 0
OFF_QVB = OFF_CRX + C * JW
OFF_PAIR = OFF_QVB + C * NA
OFF_TAILA = OFF_PAIR + 4 * PAIR
OFF_WSMALL = OFF_TAILA
OFF_W0L2 = OFF_WSMALL + 4 * SMALL
OFF_BROW = OFF_W0L2 + 2 * K
OFF_BROW2 = OFF_BROW + 4 * SMALL + W1COL + E + K
OFF_TAILB = OFF_BROW2 + 2 * E + 2 * K
OFF_W1L2 = OFF_TAILB
OFF_W2L2 = OFF_W1L2 + 2 * W1COL
NBF = OFF_W2L2 + 2 * E
# f32 mega: bias_t (8) | dmask (16) | consts (34)
GOFF_BIAS = 0
GOFF_DMASK = 8
GOFF_CONSTS = 24
GOFF_BIASN = GOFF_CONSTS + E + 2
NF32 = GOFF_BIASN + 8

_cache = {}


def _build_nc():
    nc = bacc_mod.Bacc("TRN2", target_bir_lowering=False, debug=False)

    mb_d = nc.dram_tensor("mb", [128, NBF], DT, kind="ExternalInput")
    mf_d = nc.dram_tensor("mf", [128, NF32], F32, kind="ExternalInput")
    out_d = nc.dram_tensor("out", [128, C], F32, kind="ExternalOutput")

    with tile.TileContext(nc) as tc, ExitStack() as ctx:
        pool = ctx.enter_context(tc.tile_pool(name="sbuf", bufs=1))
        hpool = ctx.enter_context(tc.tile_pool(name="hbuf", bufs=3))
        psum = ctx.enter_context(tc.tile_pool(name="psum", bufs=2, space="PSUM"))

        mb_s = pool.tile([128, NBF], DT)
        mf_s = pool.tile([128, NF32], F32)

        # ---- DMAs on both HW DGE queues; (wcat | s_t rh0) pairs first ----
        def dma_cols(eng, c0, c1):
            eng.dma_start(mb_s[:, c0:c1], mb_d[:, c0:c1])

        nc.scalar.dma_start(mf_s[:], mf_d[:])
        dma_cols(nc.sync, OFF_PAIR + 0 * PAIR, OFF_PAIR + 0 * PAIR + 1280)
        dma_cols(nc.scalar, OFF_PAIR + 1 * PAIR, OFF_PAIR + 1 * PAIR + 1280)
        dma_cols(nc.sync, OFF_PAIR + 2 * PAIR, OFF_PAIR + 2 * PAIR + 1280)
        dma_cols(nc.scalar, OFF_PAIR + 3 * PAIR, OFF_PAIR + 3 * PAIR + 1280)
        dma_cols(nc.sync, OFF_TAILB, NBF)         # w1l2 | w2l2 (w1 blocks)
        dma_cols(nc.scalar, OFF_TAILA, OFF_TAILB)  # wsmall | w0l2 | bias rows
        dma_cols(nc.sync, OFF_PAIR + 0 * PAIR + 1280, OFF_PAIR + 1 * PAIR)
        dma_cols(nc.scalar, OFF_PAIR + 1 * PAIR + 1280, OFF_PAIR + 2 * PAIR)
        dma_cols(nc.sync, OFF_PAIR + 2 * PAIR + 1280, OFF_PAIR + 3 * PAIR)
        dma_cols(nc.scalar, OFF_CRX, OFF_PAIR)     # gat | qvb (group chain)
        dma_cols(nc.scalar, OFF_PAIR + 3 * PAIR + 1280, OFF_PAIR + 4 * PAIR)

        def wcat(kc, c0, c1):
            return mb_s[:, OFF_PAIR + kc * PAIR + c0:OFF_PAIR + kc * PAIR + c1]

        def s_t(kc, c0, c1):
            return mb_s[:, OFF_PAIR + kc * PAIR + 768 + c0:
                        OFF_PAIR + kc * PAIR + 768 + c1]

        def wsmall(kc):
            return mb_s[:, OFF_WSMALL + kc * SMALL:OFF_WSMALL + (kc + 1) * SMALL]

        def w1l2(kc, c0, c1):
            return mb_s[:, OFF_W1L2 + kc * W1COL + c0:OFF_W1L2 + kc * W1COL + c1]

        def w2l2(kc):
            return mb_s[:, OFF_W2L2 + kc * E:OFF_W2L2 + (kc + 1) * E]

        def w0l2(kc):
            return mb_s[:, OFF_W0L2 + kc * K:OFF_W0L2 + (kc + 1) * K]

        acc_s = mb_s[:, OFF_CRX:OFF_CRX + C * JW].rearrange(
            "p (c j) -> p c j", j=JW)          # host-gathered q-values
        qvb_s = mb_s[:, OFF_QVB:OFF_QVB + C * NA].rearrange(
            "p (c j) -> p c j", j=NA)
        brow_small4_s = mb_s[0:1, OFF_BROW:OFF_BROW + 4 * SMALL]
        brow_w1_s = mb_s[0:1, OFF_BROW + 4 * SMALL:OFF_BROW + 4 * SMALL + W1COL]
        brow_w0_s = mb_s[0:1, OFF_BROW + 4 * SMALL + W1COL + E:
                         OFF_BROW + 4 * SMALL + W1COL + E + K]
        brow_w2x2_s = mb_s[0:1, OFF_BROW2:OFF_BROW2 + 2 * E]
        brow_w0x2_s = mb_s[0:1, OFF_BROW2 + 2 * E:OFF_BROW2 + 2 * E + 2 * K]
        bias_t_s = mf_s[:, GOFF_BIAS:GOFF_BIAS + 8]
        dmask_s = mf_s[:, GOFF_DMASK:GOFF_DMASK + NV]
        consts_s = mf_s[:, GOFF_CONSTS:GOFF_CONSTS + E + 2]
        biasn_s = mf_s[:, GOFF_BIASN:GOFF_BIASN + 8]

        # ---- PE warmup: HAM counts *array* activity, so K=1 ones-matmuls
        # do NOT lift the clock gate — use full K=128 matmuls on a memset
        # tile. The train bridges from kernel start until the first
        # wcat/s_t pair lands (~12us) so the real s1t GEMMs run at 2.4 GHz.
        # warm_s memset goes FIRST on gpsimd: it gates the train's start,
        # while ones_s isn't read until the small preload (~13us).
        def bc(ap, like):
            a, _ = broadcast_tensor_aps(ap, like)
            return a

        warm_s = pool.tile([128, 512], DT)
        nc.gpsimd.memset(warm_s[:], 0.0)

        ones_s = pool.tile([1, 512], DT)
        nc.gpsimd.memset(ones_s[:], 1.0)
        for i in range(10):
            pw = psum.tile([128, 512], F32, tag="s1r", bufs=2)
            nc.tensor.matmul(pw[:], warm_s[:, 0:128], warm_s[:],
                             start=True, stop=True)

        # ---- stage-1 transposed GEMMs: z = relu(Wcat.T @ states + b) ----
        z_s = pool.tile([128, 6, R], DT)      # zA | z1 | z2 (feat-major)
        zad_s = pool.tile([128, 2, 128], DT)  # diag zA, chunk 0 only

        def s1t_evict(p1, fc, rh):
            dst = z_s[:, fc, rh * 512:(rh + 1) * 512]
            # rh1: bias toward scalar (DVE only takes fc 3, 1) — the DVE
            # FIFO carries the mix chain for rc 0-3 in that window.
            on_dve = (fc % 2 == 0) if rh == 0 else (fc in (3, 1))
            if on_dve:
                nc.vector.scalar_tensor_tensor(
                    dst, p1[:], biasn_s[:, fc:fc + 1],
                    bc(bias_t_s[:, fc:fc + 1], p1[:]),
                    OP.max, OP.add)
            else:
                nc.scalar.activation(dst, p1[:], AF.Relu,
                                     bias=bias_t_s[:, fc:fc + 1])
            if rh == 0 and fc < 2:
                nc.scalar.activation(zad_s[:, fc, :], p1[:, 0:128],
                                     AF.Relu,
                                     bias=bias_t_s[:, 6 + fc:7 + fc])

        # kc-major over all 6 feature chunks: each arriving (wcat, s_t) DMA
        # pair unlocks 6 matmuls, so the PE FIFO never head-of-line blocks
        # on a late pair. kc 2,3 then go fc-by-fc so evictions start early,
        # prioritising z1 (fc 2,3 -> w1 blocks) then z00 (group chain).
        def s1t_alloc(rh):
            return [psum.tile([128, 512], F32, tag="s1t", bufs=6,
                              name=f"p1_{rh}_{fc}") for fc in range(6)]

        def s1t_round(rh, ps, kc):
            for fc in range(6):
                nc.tensor.matmul(
                    ps[fc][:], wcat(kc, fc * 128, (fc + 1) * 128),
                    s_t(kc, rh * 512, (rh + 1) * 512),
                    start=(kc == 0), stop=False)

        def s1t_kc23(rh, ps, extras=None):
            # extras: {fc: callable} — sparse N=4/8 blocks woven between the
            # dense N=512 groups so no HAM window drops below the duty gate.
            for fc in (2, 3, 0, 1, 4, 5):
                for kc in (2, 3):
                    nc.tensor.matmul(
                        ps[fc][:], wcat(kc, fc * 128, (fc + 1) * 128),
                        s_t(kc, rh * 512, (rh + 1) * 512),
                        start=False, stop=(kc == 3))
                s1t_evict(ps[fc], fc, rh)
                if extras and fc in extras:
                    extras[fc]()

        # ---- stage-1 row-major small heads, kc-grouped so the N=76 runs
        # interleave with the dense s1t rounds (each small-kc group needs
        # exactly the DMA pair the preceding s1t round consumed, and the
        # alternation keeps HAM array-duty above the re-throttle point).
        small_s = pool.tile([128, C, SMALL], F32)  # w01|b01|b00|b1|zb2(pre-relu)

        def small_preload(g):
            p2 = psum.tile([128, 4, SMALL], F32, tag="s1r", bufs=2)
            nc.tensor.matmul(p2[:].rearrange("p a b -> p (a b)"),
                             ones_s[:, 0:128], brow_small4_s,
                             start=True, stop=False)
            return p2

        def small_kc(g, p2, kc):
            for rl in range(4):
                rc = g * 4 + rl
                nc.tensor.matmul(p2[:, rl, :],
                                 s_t(kc, rc * 128, (rc + 1) * 128),
                                 wsmall(kc), start=False, stop=(kc == 3))
            if kc == 3:
                nc.scalar.copy(small_s[:, g * 4:(g + 1) * 4, :], p2[:])

        # ---- w0c / w0d (needs z00 only; unblocks the group chain) -------
        w0c_s = pool.tile([128, C, K], DT)
        w0d_s = pool.tile([128, K], DT)       # |w0_diag|, chunk 0

        def w0c_block(rcps):
            for rcp in rcps:
                p4 = psum.tile([128, 2, K], F32, tag="s1r", bufs=2)
                nc.tensor.matmul(p4[:].rearrange("p a b -> p (a b)"),
                                 ones_s[:, 0:128], brow_w0x2_s,
                                 start=True, stop=False)
                for rl in range(2):
                    rc = rcp * 2 + rl
                    for kc in range(2):
                        nc.tensor.matmul(p4[:, rl, :],
                                         z_s[:, 0 + kc, rc * 128:(rc + 1) * 128],
                                         w0l2(kc), start=False, stop=(kc == 1))
                nc.scalar.activation(w0c_s[:, rcp * 2:rcp * 2 + 2, :],
                                     p4[:], AF.Abs)

        def w0d_block():
            p6 = psum.tile([128, K], F32, tag="s1r", bufs=2)
            nc.tensor.matmul(p6[:], ones_s[:, 0:128], brow_w0_s,
                             start=True, stop=False)
            for kc in range(2):
                nc.tensor.matmul(p6[:], zad_s[:, kc, :], w0l2(kc),
                                 start=False, stop=(kc == 1))
            nc.scalar.activation(w0d_s[:], p6[:], AF.Abs)

        # ---- b2 head (needs only small_s / consts), per batch-half ------
        zb2r_s = pool.tile([128, C, E], F32)
        b2p_s = pool.tile([128, C, E], F32)
        b2v_s = pool.tile([128, C], F32)
        cb2 = consts_s[:, 0:E].rearrange("p (o e) -> p o e", o=1)

        def b2_head(g):
            cs = slice(g * 4, (g + 1) * 4)
            nc.scalar.activation(zb2r_s[:, cs, :], small_s[:, cs, 44:76],
                                 AF.Relu)
            nc.gpsimd.tensor_tensor(b2p_s[:, cs, :], zb2r_s[:, cs, :],
                                    bc(cb2, zb2r_s[:, cs, :]), OP.mult)
            nc.vector.tensor_reduce(b2v_s[:, cs], b2p_s[:, cs, :],
                                    AX.X, OP.add)

        # ---- group values + "other" head + gq assembly, per half --------
        gath4 = acc_s[:, :, 0:NVK].rearrange("p c (v k) -> p c v k", k=K)
        w04 = w0c_s.rearrange("p c (o k) -> p c o k", o=1)
        prodg_s = pool.tile([128, C, NV, K], DT)
        group_s = pool.tile([128, C, NV], F32)
        prodo_s = pool.tile([128, C, NA], F32)
        other_s = pool.tile([128, C], F32)
        gq_s = pool.tile([128, C, NV + 1], DT)

        def group_half(g):
            cs = slice(g * 4, (g + 1) * 4)
            nc.vector.tensor_tensor(prodg_s[:, cs], gath4[:, cs],
                                    bc(w04[:, cs], gath4[:, cs]), OP.mult)
            nc.vector.tensor_reduce(group_s[:, cs], prodg_s[:, cs],
                                    AX.X, OP.add)
            gb = small_s[:, cs, 11:12]
            nc.vector.tensor_tensor(group_s[:, cs], group_s[:, cs],
                                    bc(gb, group_s[:, cs]), OP.add)
            if g == 0:
                # diag correction (chunk 0 only)
                dw_s = pool.tile([128, K], F32)
                nc.vector.tensor_tensor(dw_s[:], w0d_s[:], w0c_s[:, 0, :],
                                        OP.subtract)
                gselp_s = pool.tile([128, K], F32)
                nc.vector.tensor_tensor(gselp_s[:], acc_s[:, 0, NVK:JW],
                                        dw_s[:], OP.mult)
                corr0_s = pool.tile([128, 1], F32)
                nc.vector.tensor_reduce(corr0_s[:], gselp_s[:], AX.X, OP.add)
                corr_s = pool.tile([128, 1], F32)
                nc.vector.tensor_scalar(corr_s[:], corr0_s[:],
                                        consts_s[:, E:E + 1], None, OP.add)
                nc.vector.scalar_tensor_tensor(group_s[:, 0, :], dmask_s,
                                               corr_s[:], group_s[:, 0, :],
                                               OP.mult, OP.add)
            nc.vector.tensor_tensor(prodo_s[:, cs], qvb_s[:, cs],
                                    small_s[:, cs, 0:NA], OP.mult)
            nc.vector.tensor_reduce(other_s[:, cs], prodo_s[:, cs],
                                    AX.X, OP.add)
            nc.vector.tensor_tensor(other_s[:, cs], other_s[:, cs],
                                    small_s[:, cs, NA], OP.add)
            nc.gpsimd.tensor_copy(gq_s[:, cs, 0:NV], group_s[:, cs])
            nc.gpsimd.tensor_copy(
                gq_s[:, cs, NV:NV + 1],
                other_s[:, cs].rearrange("p (c o) -> p c o", o=1))


        # ---- stage-2 w1 GEMMs + per-rc |w1| evictions + gq mix ----------
        # w1r cols are e-major (col = e*17+v); mix[p,rc,e] = sum_v gq*|w1|
        w1r_s = pool.tile([128, C, W1COL], DT)
        mix_s = pool.tile([128, C, E], F32)

        def w1_block(rcs):
            for rc in rcs:
                p3h = []
                for h in range(2):
                    p3 = psum.tile([128, 272], F32, tag="s1t", bufs=6, name="p3")
                    p3h.append(p3)
                    nc.tensor.matmul(p3[:], ones_s[:, 0:128],
                                     brow_w1_s[:, h * 272:(h + 1) * 272],
                                     start=True, stop=False)
                for kc in range(2):
                    for h in range(2):
                        nc.tensor.matmul(p3h[h][:],
                                         z_s[:, 2 + kc, rc * 128:(rc + 1) * 128],
                                         w1l2(kc, h * 272, (h + 1) * 272),
                                         start=False, stop=(kc == 1))
                nc.scalar.activation(w1r_s[:, rc, 0:272], p3h[0][:], AF.Abs)
                nc.scalar.activation(w1r_s[:, rc, 272:544], p3h[1][:], AF.Abs)

        def mix_prod(c0, cn, eng):
            w1v = w1r_s[:, c0:c0 + cn, :].rearrange(
                "p c (e v) -> p c e v", v=NV + 1)
            gqv = gq_s[:, c0:c0 + cn, :].rearrange(
                "p c (o v) -> p c o v", o=1)
            prodh = hpool.tile([128, cn, E, NV + 1], DT, tag="prodh")
            eng.tensor_tensor(prodh[:], w1v, bc(gqv, w1v), OP.mult)
            return prodh

        def mix_red(c0, cn, prodh):
            nc.vector.tensor_reduce(mix_s[:, c0:c0 + cn, :], prodh[:],
                                    AX.X, OP.add)

        def mix_chunk(c0, cn, eng):
            mix_red(c0, cn, mix_prod(c0, cn, eng))

        def mix_pair(c0, eng0, eng1, split_red=False):
            # one shared product tile, halves on two engines. split_red
            # reduces each half separately so the first reduce can run
            # while the second product still waits on its w1r eviction.
            prodh = hpool.tile([128, 2, E, NV + 1], DT, tag="prodh")
            for i, eng in ((0, eng0), (1, eng1)):
                w1v = w1r_s[:, c0 + i:c0 + i + 1, :].rearrange(
                    "p c (e v) -> p c e v", v=NV + 1)
                gqv = gq_s[:, c0 + i:c0 + i + 1, :].rearrange(
                    "p c (o v) -> p c o v", o=1)
                eng.tensor_tensor(prodh[:, i:i + 1], w1v, bc(gqv, w1v),
                                  OP.mult)
                if split_red:
                    nc.vector.tensor_reduce(
                        mix_s[:, c0 + i:c0 + i + 1, :], prodh[:, i:i + 1],
                        AX.X, OP.add)
            if not split_red:
                nc.vector.tensor_reduce(mix_s[:, c0:c0 + 2, :], prodh[:],
                                        AX.X, OP.add)

        w2r_s = pool.tile([128, C, E], DT)

        def w2_block(rcps):
            for rcp in rcps:
                p5 = psum.tile([128, 2, E], F32, tag="s1r", bufs=2)
                nc.tensor.matmul(p5[:].rearrange("p a b -> p (a b)"),
                                 ones_s[:, 0:128], brow_w2x2_s,
                                 start=True, stop=False)
                for rl in range(2):
                    rc = rcp * 2 + rl
                    for kc in range(2):
                        nc.tensor.matmul(p5[:, rl, :],
                                         z_s[:, 4 + kc, rc * 128:(rc + 1) * 128],
                                         w2l2(kc), start=False, stop=(kc == 1))
                nc.scalar.activation(w2r_s[:, rcp * 2:rcp * 2 + 2, :],
                                     p5[:], AF.Abs)

        # ---- hidden = elu(mix + b1), y = sum_e (hid-1)*|w2| + b2 --------
        # wsub = w2sum - b2v;  y = (ysum + b2_l2_b) - wsub
        w2sum_s = pool.tile([128, C], F32)
        wsub_s = pool.tile([128, C], F32)
        hidp_s = pool.tile([128, C, E], F32)
        m_s = pool.tile([128, C, E], F32)
        e_s = pool.tile([128, C, E], F32)
        hid_s = pool.tile([128, C, E], F32)   # = elu(hidp) + 1
        prodf_s = pool.tile([128, C, E], F32)
        ysum_s = pool.tile([128, C], F32)
        y_s = pool.tile([128, C], F32)

        # hidden = elu(hidp) computed directly: e-1 after the EXP makes the
        # -sum(w2) correction term vanish, so no w2sum/wsub reduce at all.
        def final_block(c0, cn, hidp_eng=None):
            cs = slice(c0, c0 + cn)
            (hidp_eng or nc.vector).tensor_tensor(
                hidp_s[:, cs, :], mix_s[:, cs, :],
                small_s[:, cs, 12:44], OP.add)
            nc.vector.tensor_single_scalar(m_s[:, cs, :], hidp_s[:, cs, :],
                                           0.0, OP.min)
            nc.scalar.activation(e_s[:, cs, :], m_s[:, cs, :], AF.Exp)
            nc.vector.tensor_single_scalar(m_s[:, cs, :], e_s[:, cs, :],
                                           1.0, OP.subtract)
            nc.vector.scalar_tensor_tensor(hid_s[:, cs, :], hidp_s[:, cs, :],
                                           0.0, m_s[:, cs, :], OP.max, OP.add)
            nc.vector.tensor_tensor(prodf_s[:, cs, :], hid_s[:, cs, :],
                                    w2r_s[:, cs, :], OP.mult)
            nc.vector.tensor_reduce(ysum_s[:, cs], prodf_s[:, cs, :],
                                    AX.X, OP.add)
            nc.vector.scalar_tensor_tensor(y_s[:, cs], ysum_s[:, cs],
                                           consts_s[:, E + 1:E + 2],
                                           b2v_s[:, cs], OP.add, OP.add)
            nc.sync.dma_start(out_d[:, cs], y_s[:, cs])

        # ---- phase order: rh0 pass unlocks rc 0-3 of everything, rh1
        # unlocks rc 4-7; downstream halves pipeline behind the PE stream.
        ps0 = s1t_alloc(0)
        s1t_round(0, ps0, 0)
        s1t_round(0, ps0, 1)
        s1t_kc23(0, ps0)
        # small0 after the full rh0 pass: its preload needs the TAILA DMA
        # (brow rows), which lands ~13us — interleaving it into the rounds
        # head-of-line blocks the PE FIFO on that DMA.
        p2a = small_preload(0)
        for _kc in range(4):
            small_kc(0, p2a, _kc)
        w1_block((0, 1))
        w0c_block((0, 1))       # sparse (N=4) — sandwiched between the
        w0d_block()             # dense w1 trains to keep HAM duty up
        w1_block((2, 3))
        b2_head(0)
        group_half(0)
        ph01 = mix_prod(0, 2, nc.gpsimd)
        p2b = small_preload(1)
        for _kc in range(4):
            small_kc(1, p2b, _kc)   # kc-grouped: fires as pair-b halves land
        ps1 = s1t_alloc(1)
        s1t_round(1, ps1, 0)
        s1t_round(1, ps1, 1)
        mix_chunk(2, 2, nc.vector)
        mix_red(0, 2, ph01)
        s1t_kc23(1, ps1)
        w0c_block((2, 3))
        b2_head(1)
        group_half(1)
        w1_block((4,))
        w1_block((5,))
        mix_pair(4, nc.vector, nc.vector)
        w2_block((0, 1))
        final_block(0, 4)       # runs on DVE while the PE does w1(6),(7)
        w1_block((6,))
        w2_block((2,))
        final_block(4, 2)
        w1_block((7,))
        mix_pair(6, nc.gpsimd, nc.vector, split_red=True)
        w2_block((3,))
        final_block(6, 2)

    nc.compile()
    return nc


def _prep_inputs(inputs):
    g = lambda k: np.asarray(inputs[k], dtype=np.float32)
    states = g('states')
    qvals = g('qvals')
    cr = np.asarray(inputs['causal_relations'])

    w00_l1_W, w00_l1_b = g('w00_l1_W'), g('w00_l1_b')
    b00_W, b00_b = g('b00_W'), g('b00_b')
    h_delta = w00_l1_W[SD:].sum(0)
    g_delta = float(b00_W[SD:].sum(0)[0])

    wcat = np.concatenate([w00_l1_W[:SD], g('w1_l1_W'), g('w2_l1_W')], axis=1)
    b_cat = np.concatenate([w00_l1_b, g('w1_l1_b'), g('w2_l1_b')])
    bias_t = np.zeros((128, 8), np.float32)
    for fc in range(6):
        bias_t[:, fc] = b_cat[fc * 128:(fc + 1) * 128]
    for fc in range(2):
        bias_t[:, 6 + fc] = (w00_l1_b + h_delta)[fc * 128:(fc + 1) * 128]

    wsmall = np.concatenate([g('w01_W'), g('b01_W'), b00_W[:SD],
                             g('b1_W'), g('b2_l1_W')], axis=1)
    brow_small = np.concatenate([g('w01_b'), g('b01_b'), b00_b,
                                 g('b1_b'), g('b2_l1_b')])
    perm = np.array([v * E + e for e in range(E) for v in range(NV + 1)])
    w1l2 = g('w1_l2_W')[:, perm]
    brow_w1 = g('w1_l2_b')[perm]
    w2l2, brow_w2 = g('w2_l2_W'), g('w2_l2_b')
    w0l2, brow_w0 = g('w00_l2_W'), g('w00_l2_b')

    # shared bf16 mega columns (everything except s_t / gat / qvb)
    mb_shared = np.zeros((128, NBF), BF)
    for kc in range(4):
        mb_shared[:, OFF_PAIR + kc * PAIR:OFF_PAIR + kc * PAIR + 768] = \
            wcat[kc * 128:(kc + 1) * 128]
        mb_shared[:, OFF_WSMALL + kc * SMALL:OFF_WSMALL + (kc + 1) * SMALL] = \
            wsmall[kc * 128:(kc + 1) * 128]
    for kc in range(2):
        mb_shared[:, OFF_W1L2 + kc * W1COL:OFF_W1L2 + (kc + 1) * W1COL] = \
            w1l2[kc * 128:(kc + 1) * 128]
        mb_shared[:, OFF_W2L2 + kc * E:OFF_W2L2 + (kc + 1) * E] = \
            w2l2[kc * 128:(kc + 1) * 128]
        mb_shared[:, OFF_W0L2 + kc * K:OFF_W0L2 + (kc + 1) * K] = \
            w0l2[kc * 128:(kc + 1) * 128]
    o = OFF_BROW
    mb_shared[0, o:o + 4 * SMALL] = np.tile(brow_small, 4)
    mb_shared[0, o + 4 * SMALL:o + 4 * SMALL + W1COL] = brow_w1
    mb_shared[0, o + 4 * SMALL + W1COL:o + 4 * SMALL + W1COL + E + K] = \
        np.concatenate([brow_w2, brow_w0])
    mb_shared[0, OFF_BROW2:OFF_BROW2 + 2 * E] = np.tile(brow_w2, 2)
    mb_shared[0, OFF_BROW2 + 2 * E:OFF_BROW2 + 2 * E + 2 * K] = \
        np.tile(brow_w0, 2)

    mf_shared = np.zeros((128, NF32), np.float32)
    mf_shared[:, GOFF_BIAS:GOFF_BIAS + 8] = bias_t
    mf_shared[:, GOFF_CONSTS:GOFF_CONSTS + E] = g('b2_l2_W')[:, 0][None, :]
    mf_shared[:, GOFF_CONSTS + E] = g_delta
    mf_shared[:, GOFF_CONSTS + E + 1] = float(g('b2_l2_b')[0])
    mf_shared[:, GOFF_BIASN:GOFF_BIASN + 8] = -bias_t

    to_pc = lambda x: np.ascontiguousarray(
        x.reshape(C, 128, -1).transpose(1, 0, 2).reshape(128, -1))

    in_maps = []
    for m in range(NCORES):
        bs = m + 8 * np.arange(16)
        mb = mb_shared.copy()
        S2 = states[bs].reshape(R, SD)
        s_tT = np.ascontiguousarray(S2.T).astype(BF)    # [512, R]
        for kc in range(4):
            mb[:, OFF_PAIR + kc * PAIR + 768:OFF_PAIR + (kc + 1) * PAIR] = \
                s_tT[kc * 128:(kc + 1) * 128]

        qv = qvals[bs].reshape(R, NA)
        cr_vk = np.swapaxes(cr[bs].reshape(R, K, NV), 1, 2)  # [r, v, k]
        gat = np.take_along_axis(
            np.broadcast_to(qv[:, None, :], (R, NV, NA)), cr_vk, axis=-1)
        crx = np.zeros((R, JW), np.float32)
        crx[:, 0:NVK] = gat.reshape(R, NVK)
        vd = np.where(np.arange(128) < 64, m, m + 8)
        crx[0:128, NVK:JW] = gat[np.arange(128), vd, :]
        mb[:, OFF_CRX:OFF_CRX + C * JW] = to_pc(crx)
        mb[:, OFF_QVB:OFF_QVB + C * NA] = to_pc(qv)

        mf = mf_shared.copy()
        dmask = np.zeros((128, NV), np.float32)
        dmask[np.arange(128), vd] = 1.0
        mf[:, GOFF_DMASK:GOFF_DMASK + NV] = dmask
        in_maps.append(dict(mb=mb, mf=mf))
    return in_maps


def kernel(**inputs):
    if 'nc' not in _cache:
        _cache['nc'] = _build_nc()
    nc = _cache['nc']
    in_maps = _prep_inputs(inputs)
    res = run_bass_kernel_spmd(nc, in_maps, list(range(NCORES)),
                               **_cache.get('run_kwargs', {}))
    _cache['last_result'] = res
    y = np.zeros((B, T, 1), np.float32)
    for m in range(NCORES):
        bs = m + 8 * np.arange(16)
        o = res.results[m]['out']               # [128, C]
        rows = np.ascontiguousarray(o.T).reshape(R)   # r = c*128+p
        y[bs] = rows.reshape(16, T, 1)
    return y



# revision 56
# speedup vs baseline: 1.2264x; 1.0231x over previous
"""Trainium2 Bass kernel for nn_CausalMixer (QMIX-style causal mixer).

Data-parallel across 8 NeuronCores: batch dim sharded round-robin
(core m gets batches m, m+8, m+16, ...), hypernet weights replicated.

Per-core layout (R = 1024 rows = 16 batches x 64 timesteps):
  - stage-1 "transposed" GEMMs: out[feat, rows] = Wcat.T-chunks @ states.T,
    evicted with fused per-partition bias+ReLU (alternating ScalarE / DVE).
  - stage-2 row-major GEMMs: the relu'd z tiles [feat, rows] serve directly
    as lhsT, producing per-row hypernet weights [rows, feat]; bias is
    preloaded into PSUM with a K=1 ones-matmul.
  - gather (qvals[cr]) precomputed on host, shipped in the mega tensor.
  - the onehot quirk (batch row b==v gets +delta) is handled as a rank-1
    correction on chunk 0 only (host orders the diag batches first).
  - all bf16 inputs ride one mega-packed DRAM tensor split across the two
    HW DGE queues (SP + Activation), critical (wcat,states) pairs first.

Scheduling (tuned against perfetto traces):
  - a 10-matmul K=128/N=512 warmup train on a memset tile runs from kernel
    start so the HAM clock gate flips to 8/8 (~2.4 GHz) before the real
    GEMMs; K=1 ones-matmuls do NOT register as array activity.
  - the s1t passes run kc-major across all 6 feature chunks (6 concurrent
    PSUM banks), so each arriving (wcat|s_t) DMA pair unlocks 6 matmuls
    and the PE FIFO never head-of-line blocks on a late pair; kc 2,3 then
    go fc-by-fc (z1 first) so evictions start early.
  - everything downstream is split into batch-halves (rc 0-3 / 4-7) and
    pipelined behind the PE stream; sparse N=4/N=32 blocks are sandwiched
    between dense w1 trains to keep HAM array-duty up; the w2 blocks and
    final elu/y blocks (rc 0-3 / 4-5 / 6-7) are threaded through the last
    w1 blocks so only the rc 6-7 chain gates the kernel end, with the
    last paired mix reduce split so red6 overlaps prod7's eviction wait.
"""

import sys

for _p in ("/root/.axon_site", "/root/.axon_site/_ro/trn_rl_repo",
           "/root/.axon_site/_ro/pypackages", "/opt/trn_rl_repo"):
    if _p not in sys.path:
        sys.path.append(_p)

import numpy as np
import ml_dtypes
from contextlib import ExitStack

import concourse.bass as bass
import concourse.bacc as bacc_mod
import concourse.tile as tile
import concourse.mybir as mybir
from concourse.bass import broadcast_tensor_aps
from concourse.bass_utils import run_bass_kernel_spmd

BF = ml_dtypes.bfloat16
DT = mybir.dt.bfloat16
F32 = mybir.dt.float32
OP = mybir.AluOpType
AF = mybir.ActivationFunctionType
AX = mybir.AxisListType

NCORES = 8
B, T, NA, NV, K, SD, H, E = 128, 64, 10, 16, 4, 512, 256, 32
R = 16 * T            # rows per core = 1024
C = R // 128          # row chunks per core = 8
NVK = NV * K          # 64
JW = NVK + K          # 68 gather cols (64 + 4 diag)
SMALL = NA + 1 + 1 + E + E   # 76: w01 | b01 | b00 | b1 | b2_l1
W1COL = (NV + 1) * E  # 544

# ---- mega-packed bf16 input column map (need-ordered) -------------------
# gat 8x68 | qvb 8x10, then 4x [wcat_kc (768) | s_t_kc (1024)] pairs, then:
#   wsmall 4x76 | w1l2 2x544 | w2l2 2x32 | w0l2 2x4
#   | brow_small4 4x76 | brow_w1 544 | brow_w20 36 | brow_w2x2 64
#   | brow_w0x2 8  (partition 0)
PAIR = 768 + R
OFF_CRX = 0
OFF_QVB = OFF_CRX + C * JW
OFF_PAIR = OFF_QVB + C * NA
OFF_TAILA = OFF_PAIR + 4 * PAIR
OFF_WSMALL = OFF_TAILA
OFF_W0L2 = OFF_WSMALL + 4 * SMALL
OFF_BROW = OFF_W0L2 + 2 * K
OFF_BROW2 = OFF_BROW + 4 * SMALL + W1COL + E + K
OFF_TAILB = OFF_BROW2 + 2 * E + 2 * K
OFF_W1L2 = OFF_TAILB
OFF_W2L2 = OFF_W1L2 + 2 * W1COL
NBF = OFF_W2L2 + 2 * E
# f32 mega: bias_t (8) | dmask (16) | consts (34)
GOFF_BIAS = 0
GOFF_DMASK = 8
GOFF_CONSTS = 24
GOFF_BIASN = GOFF_CONSTS + E + 2
NF32 = GOFF_BIASN + 8

_cache = {}


def _build_nc():
    nc = bacc_mod.Bacc("TRN2", target_bir_lowering=False, debug=False)

    mb_d = nc.dram_tensor("mb", [128, NBF], DT, kind="ExternalInput")
    mf_d = nc.dram_tensor("mf", [128, NF32], F32, kind="ExternalInput")
    out_d = nc.dram_tensor("out", [128, C], F32, kind="ExternalOutput")

    with tile.TileContext(nc) as tc, ExitStack() as ctx:
        pool = ctx.enter_context(tc.tile_pool(name="sbuf", bufs=1))
        hpool = ctx.enter_context(tc.tile_pool(name="hbuf", bufs=3))
        psum = ctx.enter_context(tc.tile_pool(name="psum", bufs=2, space="PSUM"))

        mb_s = pool.tile([128, NBF], DT)
        mf_s = pool.tile([128, NF32], F32)

        # ---- DMAs on both HW DGE queues; (wcat | s_t rh0) pairs first ----
        def dma_cols(eng, c0, c1):
            eng.dma_start(mb_s[:, c0:c1], mb_d[:, c0:c1])

        nc.scalar.dma_start(mf_s[:], mf_d[:])
        dma_cols(nc.sync, OFF_PAIR + 0 * PAIR, OFF_PAIR + 0 * PAIR + 1280)
        dma_cols(nc.scalar, OFF_PAIR + 1 * PAIR, OFF_PAIR + 1 * PAIR + 1280)
        dma_cols(nc.sync, OFF_PAIR + 2 * PAIR, OFF_PAIR + 2 * PAIR + 1280)
        dma_cols(nc.scalar, OFF_PAIR + 3 * PAIR, OFF_PAIR + 3 * PAIR + 1280)
        dma_cols(nc.sync, OFF_TAILB, NBF)         # w1l2 | w2l2 (w1 blocks)
        dma_cols(nc.scalar, OFF_TAILA, OFF_TAILB)  # wsmall | w0l2 | bias rows
        dma_cols(nc.sync, OFF_PAIR + 0 * PAIR + 1280, OFF_PAIR + 1 * PAIR)
        dma_cols(nc.scalar, OFF_PAIR + 1 * PAIR + 1280, OFF_PAIR + 2 * PAIR)
        dma_cols(nc.sync, OFF_PAIR + 2 * PAIR + 1280, OFF_PAIR + 3 * PAIR)
        dma_cols(nc.scalar, OFF_CRX, OFF_PAIR)     # gat | qvb (group chain)
        dma_cols(nc.scalar, OFF_PAIR + 3 * PAIR + 1280, OFF_PAIR + 4 * PAIR)

        def wcat(kc, c0, c1):
            return mb_s[:, OFF_PAIR + kc * PAIR + c0:OFF_PAIR + kc * PAIR + c1]

        def s_t(kc, c0, c1):
            return mb_s[:, OFF_PAIR + kc * PAIR + 768 + c0:
                        OFF_PAIR + kc * PAIR + 768 + c1]

        def wsmall(kc):
            return mb_s[:, OFF_WSMALL + kc * SMALL:OFF_WSMALL + (kc + 1) * SMALL]

        def w1l2(kc, c0, c1):
            return mb_s[:, OFF_W1L2 + kc * W1COL + c0:OFF_W1L2 + kc * W1COL + c1]

        def w2l2(kc):
            return mb_s[:, OFF_W2L2 + kc * E:OFF_W2L2 + (kc + 1) * E]

        def w0l2(kc):
            return mb_s[:, OFF_W0L2 + kc * K:OFF_W0L2 + (kc + 1) * K]

        acc_s = mb_s[:, OFF_CRX:OFF_CRX + C * JW].rearrange(
            "p (c j) -> p c j", j=JW)          # host-gathered q-values
        qvb_s = mb_s[:, OFF_QVB:OFF_QVB + C * NA].rearrange(
            "p (c j) -> p c j", j=NA)
        brow_small4_s = mb_s[0:1, OFF_BROW:OFF_BROW + 4 * SMALL]
        brow_w1_s = mb_s[0:1, OFF_BROW + 4 * SMALL:OFF_BROW + 4 * SMALL + W1COL]
        brow_w0_s = mb_s[0:1, OFF_BROW + 4 * SMALL + W1COL + E:
                         OFF_BROW + 4 * SMALL + W1COL + E + K]
        brow_w2x2_s = mb_s[0:1, OFF_BROW2:OFF_BROW2 + 2 * E]
        brow_w0x2_s = mb_s[0:1, OFF_BROW2 + 2 * E:OFF_BROW2 + 2 * E + 2 * K]
        bias_t_s = mf_s[:, GOFF_BIAS:GOFF_BIAS + 8]
        dmask_s = mf_s[:, GOFF_DMASK:GOFF_DMASK + NV]
        consts_s = mf_s[:, GOFF_CONSTS:GOFF_CONSTS + E + 2]
        biasn_s = mf_s[:, GOFF_BIASN:GOFF_BIASN + 8]

        # ---- PE warmup: HAM counts *array* activity, so K=1 ones-matmuls
        # do NOT lift the clock gate — use full K=128 matmuls on a memset
        # tile. The train bridges from kernel start until the first
        # wcat/s_t pair lands (~12us) so the real s1t GEMMs run at 2.4 GHz.
        # warm_s memset goes FIRST on gpsimd: it gates the train's start,
        # while ones_s isn't read until the small preload (~13us).
        def bc(ap, like):
            a, _ = broadcast_tensor_aps(ap, like)
            return a

        warm_s = pool.tile([128, 512], DT)
        nc.gpsimd.memset(warm_s[:], 0.0)

        ones_s = pool.tile([1, 512], DT)
        nc.gpsimd.memset(ones_s[:], 1.0)
        for i in range(10):
            pw = psum.tile([128, 512], F32, tag="s1r", bufs=2)
            nc.tensor.matmul(pw[:], warm_s[:, 0:128], warm_s[:],
                             start=True, stop=True)

        # ---- stage-1 transposed GEMMs: z = relu(Wcat.T @ states + b) ----
        z_s = pool.tile([128, 6, R], DT)      # zA | z1 | z2 (feat-major)
        zad_s = pool.tile([128, 2, 128], DT)  # diag zA, chunk 0 only

        def s1t_evict(p1, fc, rh):
            dst = z_s[:, fc, rh * 512:(rh + 1) * 512]
            # rh1: bias toward scalar (DVE only takes fc 3, 1) — the DVE
            # FIFO carries the mix chain for rc 0-3 in that window.
            on_dve = (fc % 2 == 0) if rh == 0 else (fc in (3, 1))
            if on_dve:
                nc.vector.scalar_tensor_tensor(
                    dst, p1[:], biasn_s[:, fc:fc + 1],
                    bc(bias_t_s[:, fc:fc + 1], p1[:]),
                    OP.max, OP.add)
            else:
                nc.scalar.activation(dst, p1[:], AF.Relu,
                                     bias=bias_t_s[:, fc:fc + 1])
            if rh == 0 and fc < 2:
                nc.scalar.activation(zad_s[:, fc, :], p1[:, 0:128],
                                     AF.Relu,
                                     bias=bias_t_s[:, 6 + fc:7 + fc])

        # kc-major over all 6 feature chunks: each arriving (wcat, s_t) DMA
        # pair unlocks 6 matmuls, so the PE FIFO never head-of-line blocks
        # on a late pair. kc 2,3 then go fc-by-fc so evictions start early,
        # prioritising z1 (fc 2,3 -> w1 blocks) then z00 (group chain).
        def s1t_alloc(rh):
            return [psum.tile([128, 512], F32, tag="s1t", bufs=6,
                              name=f"p1_{rh}_{fc}") for fc in range(6)]

        def s1t_round(rh, ps, kc):
            for fc in range(6):
                nc.tensor.matmul(
                    ps[fc][:], wcat(kc, fc * 128, (fc + 1) * 128),
                    s_t(kc, rh * 512, (rh + 1) * 512),
                    start=(kc == 0), stop=False)

        def s1t_kc23(rh, ps, extras=None):
            # extras: {fc: callable} — sparse N=4/8 blocks woven between the
            # dense N=512 groups so no HAM window drops below the duty gate.
            for fc in (2, 3, 0, 1, 4, 5):
                for kc in (2, 3):
                    nc.tensor.matmul(
                        ps[fc][:], wcat(kc, fc * 128, (fc + 1) * 128),
                        s_t(kc, rh * 512, (rh + 1) * 512),
                        start=False, stop=(kc == 3))
                s1t_evict(ps[fc], fc, rh)
                if extras and fc in extras:
                    extras[fc]()

        # ---- stage-1 row-major small heads, kc-grouped so the N=76 runs
        # interleave with the dense s1t rounds (each small-kc group needs
        # exactly the DMA pair the preceding s1t round consumed, and the
        # alternation keeps HAM array-duty above the re-throttle point).
        small_s = pool.tile([128, C, SMALL], F32)  # w01|b01|b00|b1|zb2(pre-relu)

        def small_preload(g):
            p2 = psum.tile([128, 4, SMALL], F32, tag="s1r", bufs=2)
            nc.tensor.matmul(p2[:].rearrange("p a b -> p (a b)"),
                             ones_s[:, 0:128], brow_small4_s,
                             start=True, stop=False)
            return p2

        def small_kc(g, p2, kc):
            for rl in range(4):
                rc = g * 4 + rl
                nc.tensor.matmul(p2[:, rl, :],
                                 s_t(kc, rc * 128, (rc + 1) * 128),
                                 wsmall(kc), start=False, stop=(kc == 3))
            if kc == 3:
                nc.scalar.copy(small_s[:, g * 4:(g + 1) * 4, :], p2[:])

        # ---- w0c / w0d (needs z00 only; unblocks the group chain) -------
        w0c_s = pool.tile([128, C, K], DT)
        w0d_s = pool.tile([128, K], DT)       # |w0_diag|, chunk 0

        def w0c_block(rcps):
            for rcp in rcps:
                p4 = psum.tile([128, 2, K], F32, tag="s1r", bufs=2)
                nc.tensor.matmul(p4[:].rearrange("p a b -> p (a b)"),
                                 ones_s[:, 0:128], brow_w0x2_s,
                                 start=True, stop=False)
                for rl in range(2):
                    rc = rcp * 2 + rl
                    for kc in range(2):
                        nc.tensor.matmul(p4[:, rl, :],
                                         z_s[:, 0 + kc, rc * 128:(rc + 1) * 128],
                                         w0l2(kc), start=False, stop=(kc == 1))
                nc.scalar.activation(w0c_s[:, rcp * 2:rcp * 2 + 2, :],
                                     p4[:], AF.Abs)

        def w0d_block():
            p6 = psum.tile([128, K], F32, tag="s1r", bufs=2)
            nc.tensor.matmul(p6[:], ones_s[:, 0:128], brow_w0_s,
                             start=True, stop=False)
            for kc in range(2):
                nc.tensor.matmul(p6[:], zad_s[:, kc, :], w0l2(kc),
                                 start=False, stop=(kc == 1))
            nc.scalar.activation(w0d_s[:], p6[:], AF.Abs)

        # ---- b2 head (needs only small_s / consts), per batch-half ------
        zb2r_s = pool.tile([128, C, E], F32)
        b2p_s = pool.tile([128, C, E], F32)
        b2v_s = pool.tile([128, C], F32)
        cb2 = consts_s[:, 0:E].rearrange("p (o e) -> p o e", o=1)

        def b2_head(g):
            cs = slice(g * 4, (g + 1) * 4)
            nc.scalar.activation(zb2r_s[:, cs, :], small_s[:, cs, 44:76],
                                 AF.Relu)
            nc.gpsimd.tensor_tensor(b2p_s[:, cs, :], zb2r_s[:, cs, :],
                                    bc(cb2, zb2r_s[:, cs, :]), OP.mult)
            nc.vector.tensor_reduce(b2v_s[:, cs], b2p_s[:, cs, :],
                                    AX.X, OP.add)

        # ---- group values + "other" head + gq assembly, per half --------
        gath4 = acc_s[:, :, 0:NVK].rearrange("p c (v k) -> p c v k", k=K)
        w04 = w0c_s.rearrange("p c (o k) -> p c o k", o=1)
        prodg_s = pool.tile([128, C, NV, K], DT)
        group_s = pool.tile([128, C, NV], F32)
        prodo_s = pool.tile([128, C, NA], F32)
        other_s = pool.tile([128, C], F32)
        gq_s = pool.tile([128, C, NV + 1], DT)

        def group_half(g):
            cs = slice(g * 4, (g + 1) * 4)
            nc.vector.tensor_tensor(prodg_s[:, cs], gath4[:, cs],
                                    bc(w04[:, cs], gath4[:, cs]), OP.mult)
            nc.vector.tensor_reduce(group_s[:, cs], prodg_s[:, cs],
                                    AX.X, OP.add)
            gb = small_s[:, cs, 11:12]
            nc.vector.tensor_tensor(group_s[:, cs], group_s[:, cs],
                                    bc(gb, group_s[:, cs]), OP.add)
            if g == 0:
                # diag correction (chunk 0 only)
                dw_s = pool.tile([128, K], F32)
                nc.vector.tensor_tensor(dw_s[:], w0d_s[:], w0c_s[:, 0, :],
                                        OP.subtract)
                gselp_s = pool.tile([128, K], F32)
                nc.vector.tensor_tensor(gselp_s[:], acc_s[:, 0, NVK:JW],
                                        dw_s[:], OP.mult)
                corr0_s = pool.tile([128, 1], F32)
                nc.vector.tensor_reduce(corr0_s[:], gselp_s[:], AX.X, OP.add)
                corr_s = pool.tile([128, 1], F32)
                nc.vector.tensor_scalar(corr_s[:], corr0_s[:],
                                        consts_s[:, E:E + 1], None, OP.add)
                nc.vector.scalar_tensor_tensor(group_s[:, 0, :], dmask_s,
                                               corr_s[:], group_s[:, 0, :],
                                               OP.mult, OP.add)
            nc.vector.tensor_tensor(prodo_s[:, cs], qvb_s[:, cs],
                                    small_s[:, cs, 0:NA], OP.mult)
            nc.vector.tensor_reduce(other_s[:, cs], prodo_s[:, cs],
                                    AX.X, OP.add)
            nc.vector.tensor_tensor(other_s[:, cs], other_s[:, cs],
                                    small_s[:, cs, NA], OP.add)
            nc.gpsimd.tensor_copy(gq_s[:, cs, 0:NV], group_s[:, cs])
            nc.gpsimd.tensor_copy(
                gq_s[:, cs, NV:NV + 1],
                other_s[:, cs].rearrange("p (c o) -> p c o", o=1))


        # ---- stage-2 w1 GEMMs + per-rc |w1| evictions + gq mix ----------
        # w1r cols are e-major (col = e*17+v); mix[p,rc,e] = sum_v gq*|w1|
        w1r_s = pool.tile([128, C, W1COL], DT)
        mix_s = pool.tile([128, C, E], F32)

        def w1_block(rcs):
            for rc in rcs:
                p3h = []
                for h in range(2):
                    p3 = psum.tile([128, 272], F32, tag="s1t", bufs=6, name="p3")
                    p3h.append(p3)
                    nc.tensor.matmul(p3[:], ones_s[:, 0:128],
                                     brow_w1_s[:, h * 272:(h + 1) * 272],
                                     start=True, stop=False)
                for kc in range(2):
                    for h in range(2):
                        nc.tensor.matmul(p3h[h][:],
                                         z_s[:, 2 + kc, rc * 128:(rc + 1) * 128],
                                         w1l2(kc, h * 272, (h + 1) * 272),
                                         start=False, stop=(kc == 1))
                nc.scalar.activation(w1r_s[:, rc, 0:272], p3h[0][:], AF.Abs)
                nc.scalar.activation(w1r_s[:, rc, 272:544], p3h[1][:], AF.Abs)

        def mix_prod(c0, cn, eng):
            w1v = w1r_s[:, c0:c0 + cn, :].rearrange(
                "p c (e v) -> p c e v", v=NV + 1)
            gqv = gq_s[:, c0:c0 + cn, :].rearrange(
                "p c (o v) -> p c o v", o=1)
            prodh = hpool.tile([128, cn, E, NV + 1], DT, tag="prodh")
            eng.tensor_tensor(prodh[:], w1v, bc(gqv, w1v), OP.mult)
            return prodh

        def mix_red(c0, cn, prodh):
            nc.vector.tensor_reduce(mix_s[:, c0:c0 + cn, :], prodh[:],
                                    AX.X, OP.add)

        def mix_chunk(c0, cn, eng):
            mix_red(c0, cn, mix_prod(c0, cn, eng))

        def mix_pair(c0, eng0, eng1, split_red=False):
            # one shared product tile, halves on two engines. split_red
            # reduces each half separately so the first reduce can run
            # while the second product still waits on its w1r eviction.
            prodh = hpool.tile([128, 2, E, NV + 1], DT, tag="prodh")
            for i, eng in ((0, eng0), (1, eng1)):
                w1v = w1r_s[:, c0 + i:c0 + i + 1, :].rearrange(
                    "p c (e v) -> p c e v", v=NV + 1)
                gqv = gq_s[:, c0 + i:c0 + i + 1, :].rearrange(
                    "p c (o v) -> p c o v", o=1)
                eng.tensor_tensor(prodh[:, i:i + 1], w1v, bc(gqv, w1v),
                                  OP.mult)
                if split_red:
                    nc.vector.tensor_reduce(
                        mix_s[:, c0 + i:c0 + i + 1, :], prodh[:, i:i + 1],
                        AX.X, OP.add)
            if not split_red:
                nc.vector.tensor_reduce(mix_s[:, c0:c0 + 2, :], prodh[:],
                                        AX.X, OP.add)

        w2r_s = pool.tile([128, C, E], DT)

        def w2_block(rcps):
            for rcp in rcps:
                p5 = psum.tile([128, 2, E], F32, tag="s1r", bufs=2)
                nc.tensor.matmul(p5[:].rearrange("p a b -> p (a b)"),
                                 ones_s[:, 0:128], brow_w2x2_s,
                                 start=True, stop=False)
                for rl in range(2):
                    rc = rcp * 2 + rl
                    for kc in range(2):
                        nc.tensor.matmul(p5[:, rl, :],
                                         z_s[:, 4 + kc, rc * 128:(rc + 1) * 128],
                                         w2l2(kc), start=False, stop=(kc == 1))
                nc.scalar.activation(w2r_s[:, rcp * 2:rcp * 2 + 2, :],
                                     p5[:], AF.Abs)

        # ---- hidden = elu(mix + b1), y = sum_e (hid-1)*|w2| + b2 --------
        # wsub = w2sum - b2v;  y = (ysum + b2_l2_b) - wsub
        w2sum_s = pool.tile([128, C], F32)
        wsub_s = pool.tile([128, C], F32)
        hidp_s = pool.tile([128, C, E], F32)
        m_s = pool.tile([128, C, E], F32)
        e_s = pool.tile([128, C, E], F32)
        hid_s = pool.tile([128, C, E], F32)   # = elu(hidp) + 1
        prodf_s = pool.tile([128, C, E], F32)
        ysum_s = pool.tile([128, C], F32)
        y_s = pool.tile([128, C], F32)

        # hidden = elu(hidp) computed directly: e-1 after the EXP makes the
        # -sum(w2) correction term vanish, so no w2sum/wsub reduce at all.
        def final_block(c0, cn, hidp_eng=None):
            cs = slice(c0, c0 + cn)
            (hidp_eng or nc.vector).tensor_tensor(
                hidp_s[:, cs, :], mix_s[:, cs, :],
                small_s[:, cs, 12:44], OP.add)
            nc.vector.tensor_single_scalar(m_s[:, cs, :], hidp_s[:, cs, :],
                                           0.0, OP.min)
            nc.scalar.activation(e_s[:, cs, :], m_s[:, cs, :], AF.Exp)
            nc.vector.tensor_single_scalar(m_s[:, cs, :], e_s[:, cs, :],
                                           1.0, OP.subtract)
            nc.vector.scalar_tensor_tensor(hid_s[:, cs, :], hidp_s[:, cs, :],
                                           0.0, m_s[:, cs, :], OP.max, OP.add)
            nc.vector.tensor_tensor(prodf_s[:, cs, :], hid_s[:, cs, :],
                                    w2r_s[:, cs, :], OP.mult)
            nc.vector.tensor_reduce(ysum_s[:, cs], prodf_s[:, cs, :],
                                    AX.X, OP.add)
            nc.vector.scalar_tensor_tensor(y_s[:, cs], ysum_s[:, cs],
                                           consts_s[:, E + 1:E + 2],
                                           b2v_s[:, cs], OP.add, OP.add)
            nc.sync.dma_start(out_d[:, cs], y_s[:, cs])

        # ---- phase order: rh0 pass unlocks rc 0-3 of everything, rh1
        # unlocks rc 4-7; downstream halves pipeline behind the PE stream.
        ps0 = s1t_alloc(0)
        s1t_round(0, ps0, 0)
        s1t_round(0, ps0, 1)
        # sparse w0c/w0d woven after the LAST two dense kc23 groups: TAILA
        # (their bias rows) lands right behind pair3a on the same queue, so
        # by fc4/fc5 it is present in every run and cannot FIFO-block; this
        # keeps the w1-train region free of low-duty bursts (HAM stays warm)
        # and starts the group-g0 chain earlier.
        s1t_kc23(0, ps0, extras={
            4: lambda: w0c_block((0,)),
            5: lambda: (w0c_block((1,)), w0d_block()),
        })
        # small0 after the full rh0 pass: its preload needs the TAILA DMA
        # (brow rows), which lands ~13us — interleaving it into the rounds
        # head-of-line blocks the PE FIFO on that DMA.
        p2a = small_preload(0)
        for _kc in range(4):
            small_kc(0, p2a, _kc)
        w1_block((0, 1))
        w1_block((2, 3))
        b2_head(0)
        group_half(0)
        ph01 = mix_prod(0, 2, nc.gpsimd)
        p2b = small_preload(1)
        for _kc in range(4):
            small_kc(1, p2b, _kc)   # kc-grouped: fires as pair-b halves land
        ps1 = s1t_alloc(1)
        s1t_round(1, ps1, 0)
        s1t_round(1, ps1, 1)
        mix_chunk(2, 2, nc.vector)
        mix_red(0, 2, ph01)
        s1t_kc23(1, ps1)
        w0c_block((2, 3))
        b2_head(1)
        group_half(1)
        w1_block((4,))
        w1_block((5,))
        mix_pair(4, nc.vector, nc.vector)
        w2_block((0, 1))
        final_block(0, 4)       # runs on DVE while the PE does w1(6),(7)
        w1_block((6,))
        w2_block((2,))
        final_block(4, 2)
        w1_block((7,))
        mix_pair(6, nc.gpsimd, nc.vector, split_red=True)
        w2_block((3,))
        final_block(6, 2)

    nc.compile()
    return nc


def _prep_inputs(inputs):
    g = lambda k: np.asarray(inputs[k], dtype=np.float32)
    states = g('states')
    qvals = g('qvals')
    cr = np.asarray(inputs['causal_relations'])

    w00_l1_W, w00_l1_b = g('w00_l1_W'), g('w00_l1_b')
    b00_W, b00_b = g('b00_W'), g('b00_b')
    h_delta = w00_l1_W[SD:].sum(0)
    g_delta = float(b00_W[SD:].sum(0)[0])

    wcat = np.concatenate([w00_l1_W[:SD], g('w1_l1_W'), g('w2_l1_W')], axis=1)
    b_cat = np.concatenate([w00_l1_b, g('w1_l1_b'), g('w2_l1_b')])
    bias_t = np.zeros((128, 8), np.float32)
    for fc in range(6):
        bias_t[:, fc] = b_cat[fc * 128:(fc + 1) * 128]
    for fc in range(2):
        bias_t[:, 6 + fc] = (w00_l1_b + h_delta)[fc * 128:(fc + 1) * 128]

    wsmall = np.concatenate([g('w01_W'), g('b01_W'), b00_W[:SD],
                             g('b1_W'), g('b2_l1_W')], axis=1)
    brow_small = np.concatenate([g('w01_b'), g('b01_b'), b00_b,
                                 g('b1_b'), g('b2_l1_b')])
    perm = np.array([v * E + e for e in range(E) for v in range(NV + 1)])
    w1l2 = g('w1_l2_W')[:, perm]
    brow_w1 = g('w1_l2_b')[perm]
    w2l2, brow_w2 = g('w2_l2_W'), g('w2_l2_b')
    w0l2, brow_w0 = g('w00_l2_W'), g('w00_l2_b')

    # shared bf16 mega columns (everything except s_t / gat / qvb)
    mb_shared = np.zeros((128, NBF), BF)
    for kc in range(4):
        mb_shared[:, OFF_PAIR + kc * PAIR:OFF_PAIR + kc * PAIR + 768] = \
            wcat[kc * 128:(kc + 1) * 128]
        mb_shared[:, OFF_WSMALL + kc * SMALL:OFF_WSMALL + (kc + 1) * SMALL] = \
            wsmall[kc * 128:(kc + 1) * 128]
    for kc in range(2):
        mb_shared[:, OFF_W1L2 + kc * W1COL:OFF_W1L2 + (kc + 1) * W1COL] = \
            w1l2[kc * 128:(kc + 1) * 128]
        mb_shared[:, OFF_W2L2 + kc * E:OFF_W2L2 + (kc + 1) * E] = \
            w2l2[kc * 128:(kc + 1) * 128]
        mb_shared[:, OFF_W0L2 + kc * K:OFF_W0L2 + (kc + 1) * K] = \
            w0l2[kc * 128:(kc + 1) * 128]
    o = OFF_BROW
    mb_shared[0, o:o + 4 * SMALL] = np.tile(brow_small, 4)
    mb_shared[0, o + 4 * SMALL:o + 4 * SMALL + W1COL] = brow_w1
    mb_shared[0, o + 4 * SMALL + W1COL:o + 4 * SMALL + W1COL + E + K] = \
        np.concatenate([brow_w2, brow_w0])
    mb_shared[0, OFF_BROW2:OFF_BROW2 + 2 * E] = np.tile(brow_w2, 2)
    mb_shared[0, OFF_BROW2 + 2 * E:OFF_BROW2 + 2 * E + 2 * K] = \
        np.tile(brow_w0, 2)

    mf_shared = np.zeros((128, NF32), np.float32)
    mf_shared[:, GOFF_BIAS:GOFF_BIAS + 8] = bias_t
    mf_shared[:, GOFF_CONSTS:GOFF_CONSTS + E] = g('b2_l2_W')[:, 0][None, :]
    mf_shared[:, GOFF_CONSTS + E] = g_delta
    mf_shared[:, GOFF_CONSTS + E + 1] = float(g('b2_l2_b')[0])
    mf_shared[:, GOFF_BIASN:GOFF_BIASN + 8] = -bias_t

    to_pc = lambda x: np.ascontiguousarray(
        x.reshape(C, 128, -1).transpose(1, 0, 2).reshape(128, -1))

    in_maps = []
    for m in range(NCORES):
        bs = m + 8 * np.arange(16)
        mb = mb_shared.copy()
        S2 = states[bs].reshape(R, SD)
        s_tT = np.ascontiguousarray(S2.T).astype(BF)    # [512, R]
        for kc in range(4):
            mb[:, OFF_PAIR + kc * PAIR + 768:OFF_PAIR + (kc + 1) * PAIR] = \
                s_tT[kc * 128:(kc + 1) * 128]

        qv = qvals[bs].reshape(R, NA)
        cr_vk = np.swapaxes(cr[bs].reshape(R, K, NV), 1, 2)  # [r, v, k]
        gat = np.take_along_axis(
            np.broadcast_to(qv[:, None, :], (R, NV, NA)), cr_vk, axis=-1)
        crx = np.zeros((R, JW), np.float32)
        crx[:, 0:NVK] = gat.reshape(R, NVK)
        vd = np.where(np.arange(128) < 64, m, m + 8)
        crx[0:128, NVK:JW] = gat[np.arange(128), vd, :]
        mb[:, OFF_CRX:OFF_CRX + C * JW] = to_pc(crx)
        mb[:, OFF_QVB:OFF_QVB + C * NA] = to_pc(qv)

        mf = mf_shared.copy()
        dmask = np.zeros((128, NV), np.float32)
        dmask[np.arange(128), vd] = 1.0
        mf[:, GOFF_DMASK:GOFF_DMASK + NV] = dmask
        in_maps.append(dict(mb=mb, mf=mf))
    return in_maps


def kernel(**inputs):
    if 'nc' not in _cache:
        _cache['nc'] = _build_nc()
    nc = _cache['nc']
    in_maps = _prep_inputs(inputs)
    res = run_bass_kernel_spmd(nc, in_maps, list(range(NCORES)),
                               **_cache.get('run_kwargs', {}))
    _cache['last_result'] = res
    y = np.zeros((B, T, 1), np.float32)
    for m in range(NCORES):
        bs = m + 8 * np.arange(16)
        o = res.results[m]['out']               # [128, C]
        rows = np.ascontiguousarray(o.T).reshape(R)   # r = c*128+p
        y[bs] = rows.reshape(16, T, 1)
    return y

